# revision 46
# baseline (speedup 1.0000x reference)
"""Trainium2 Bass kernel for nn_MixtureOfExpertsLayer (moe_routing).

Sparse dispatch: top-2 routing is computed on the host (the router is a
tiny [8192,1024]@[1024,4] GEMM); tokens are gathered per expert and
sharded across the 8 cores so each core runs a fixed 512-token slab
through each of the 4 experts — half the dense FLOPs.  The linear
chains inside experts 1/2 are pre-folded on the host
(eq_w@wv@wo and syn_w@(I+wv@wo)), removing another ~11% of matmul work.

Device compute is bf16 (PSUM accumulates fp32).  Weights are pre-packed
on the host into the exact [p, kc, 256-col] tile layout the tensor
engine wants, so every DMA is a fully contiguous 0.5-2MB block.
Activations stay feature-major [128p, chunk, tok]; every matmul has a
512-token moving dim (full PE rate).  Expert outputs come back
feature-major [H, 512] fp32; the host applies the top-2 softmax gates
and scatter-adds into the final output.  Tokens beyond the
4096-per-expert device capacity (a few dozen when routing is balanced)
are computed on the host in fp64.

PE-roofline tuning (vs the first working version):
 - LayerNorm stats no longer use PE matmuls: per-chunk sums/squares are
   tree-summed on the DVE and reduced across partitions with the
   GpSimd daisy-chain partition_all_reduce (result already broadcast),
   freeing ~8us of tensor-engine time and two PSUM pools.
 - The first E0 weight/activation DMAs are split into quarter-chunks so
   the first matmul starts as soon as ~0.4MB (not 2MB) has landed, and
   a short burst of dummy matmuls warms the PE HAM clock-gate during
   the initial DMA fill (cold PE runs at 1.2GHz, warm at 2.4GHz).
 - Output-eviction DMAs are issued from the (otherwise idle) GpSimd
   DGE so weight-prefetch issue on the Sync engine is never queued
   behind them; the matmul PSUM pool gets 6 banks.
"""
import math

import numpy as np
import ml_dtypes

import concourse.bass as bass
import concourse.mybir as mybir
import concourse.tile as tile
from concourse import bacc
from concourse.alu_op_type import AluOpType
from concourse.bass_utils import run_bass_kernel_spmd

F32 = mybir.dt.float32
BF16 = mybir.dt.bfloat16
FP8 = mybir.dt.float8e4
ACT = mybir.ActivationFunctionType
AX = mybir.AxisListType
OP = AluOpType
RED = bass.bass_isa.ReduceOp
DR = mybir.MatmulPerfMode.DoubleRow
BF = ml_dtypes.bfloat16
F8 = ml_dtypes.float8_e4m3

N_CORES = 8
B, S, H, I, E = 4, 2048, 1024, 4096, 4
P = 128
T = 512                   # tokens per expert per core
CAP = N_CORES * T         # device capacity per expert
KC = H // P               # 8
KI = I // P               # 32
K2 = (2 * H) // P         # 16

# packed weight dram tensors: name -> (n_256col_blocks, contraction_chunks)
PACKED_W = {
    "w1p": (I // 256, KC), "w3p": (I // 256, KC), "m1p": (I // 256, KC),
    "w2p": (H // 256, KI), "m2p": (H // 256, KI),
    "c1p": (2 * H // 256, KC), "f1p": (2 * H // 256, KC),
    "c2p": (H // 256, K2), "f2p": (H // 256, K2),
    "a2p": (H // 256, KC), "genp": (H // 256, KC),
}
# expert 1 and E0's down-projection run in fp8-e4m3 with DoubleRow
# (simulated total error 0.0147 vs the 2e-2 gate); weights are
# pre-scaled by a power of two on the host and the inverse scales ride
# in the bias pack
FP8_W = {"c1p", "c2p", "w2p"}
FP8_OUT_SCALE = {"c2p": "c2s", "w2p": "w2s"}
# biases live in one packed [P, sum] f32 tensor; name -> n_chunks
BIASES = {
    "c1b": K2, "c2b": KC,
    "a2b": KC, "f1b": K2, "f2b": KC,
    "n1g": KC, "n1b": KC, "n2g": KC, "n2b": KC, "genb": KC,
    "m1b": KI, "m2b": KC, "zb": KC,
    "c1s": 1, "c2s": 1, "w2s": 1, "hsc": 1,
}
BIAS_OFF = {}
_off = 0
for _n, _c in BIASES.items():
    BIAS_OFF[_n] = _off
    _off += _c
BIAS_COLS = _off


def build_moe_sparse():
    nc = bacc.Bacc("TRN2", target_bir_lowering=False, debug=False)

    xg = [nc.dram_tensor(f"xg{e}", [P, KC, T], FP8 if e == 1 else BF16,
                         kind="ExternalInput")
          for e in range(E)]
    wd = {n: nc.dram_tensor(n, [nb, P, kcc, 256],
                            FP8 if n in FP8_W else BF16,
                            kind="ExternalInput")
          for n, (nb, kcc) in PACKED_W.items()}
    ball = nc.dram_tensor("ball", [P, BIAS_COLS], F32, kind="ExternalInput")
    ys = [nc.dram_tensor(f"y{e}", [P, KC, T], F32, kind="ExternalOutput")
          for e in range(E)]

    with tile.TileContext(nc) as tc:
        with (
            tc.tile_pool(name="const", bufs=1) as cpool,
            tc.tile_pool(name="xg", bufs=2) as xpool,
            tc.tile_pool(name="h1", bufs=1) as hpool,
            tc.tile_pool(name="inter", bufs=1) as ipool,
            tc.tile_pool(name="ws", bufs=4) as wsp,     # KC-contraction blocks
            tc.tile_pool(name="ws2", bufs=3) as wsp2,   # K2-contraction blocks
            tc.tile_pool(name="wb", bufs=2) as wbp,     # KI-contraction blocks
            tc.tile_pool(name="yev", bufs=2) as ypool,  # output eviction
            tc.tile_pool(name="lns", bufs=2) as lnsp,   # LN stat tiles
            tc.tile_pool(name="sq", bufs=2) as sqp,     # LN scratch (bf16)
            tc.tile_pool(name="ps", bufs=7, space=bass.MemorySpace.PSUM) as psp,
            tc.tile_pool(name="psw", bufs=1, space=bass.MemorySpace.PSUM) as pswp,
        ):
            # ---- constants ------------------------------------------------
            ones_c = cpool.tile([P, 1], BF16, tag="ones_c")
            nc.vector.memset(ones_c[:], 1.0)
            dmt = cpool.tile([P, T], BF16, tag="dmt")
            nc.vector.memset(dmt[:], 0.0078125)

            bt_all = cpool.tile([P, BIAS_COLS], F32, tag="ball")
            bt = {n: bt_all[:, BIAS_OFF[n]:BIAS_OFF[n] + nch]
                  for n, nch in BIASES.items()}

            # ---- HAM warm-up: dummy matmuls fill the initial DMA wait ----
            wps = pswp.tile([1, T], F32, tag="warm")

            def warm(n):
                # dummy matmuls on resident constants: keep the PE HAM
                # activity window fed while the startup DMAs stream in
                for _ in range(n):
                    nc.tensor.matmul(wps[:], ones_c[:], dmt[:],
                                     start=True, stop=True)

            # a short contiguous burst while the first DMAs stream in; the
            # startup is ring-throughput-bound, so the early real matmuls
            # run at the cold-clock pace that matches the ring anyway
            warm(7)

            # E0's intermediate is written directly in scaled fp8 for the
            # w2p DoubleRow down-projection; E3 later reuses the same pool
            # slot for its bf16 intermediate (disjoint lifetimes)
            h1f = hpool.tile([P, KI, T], FP8, tag="h1", name="h1f")
            # bias/scale pack: tiny (36KB) — load first on the idle gpsimd
            # DGE so the fp8 scale columns are resident before E0's first
            # h1 write
            nc.gpsimd.dma_start(bt_all[:], ball.ap())

            # ---- helpers --------------------------------------------------
            def load_xg(e, split=1):
                t_ = xpool.tile([P, KC, T], FP8 if e == 1 else BF16,
                                tag="xg", name=f"xgt{e}")
                step = KC // split
                for i in range(split):
                    sl = slice(i * step, (i + 1) * step)
                    nc.sync.dma_start(t_[:, sl, :], xg[e].ap()[:, sl, :])
                return t_

            def mm_block(ps, wc, src, src_kc, ml, fp8):
                """Accumulate one 256-col block into ps; fp8 runs DoubleRow
                (two contraction chunks per instruction at 2x rate)."""
                if fp8:
                    for kc in range(0, src_kc, 2):
                        nc.tensor.matmul(
                            ps[:], wc[:, kc:kc + 2, ml * P:(ml + 1) * P],
                            src[:, kc:kc + 2, :],
                            start=(kc == 0), stop=(kc == src_kc - 2),
                            perf_mode=DR)
                else:
                    for kc in range(src_kc):
                        nc.tensor.matmul(
                            ps[:], wc[:, kc, ml * P:(ml + 1) * P],
                            src[:, kc, :],
                            start=(kc == 0), stop=(kc == src_kc - 1))

            def up_proj(dst, wname, src, src_kc, act, bias, blocks=None,
                        pool=None, scale=None, w0=None, hook=None):
                """dst[:, c, :] = act(scale * (Wc.T @ src) + bias_c),
                streamed in 256-col blocks.  dst chunk c = 2*b + ml."""
                pool = pool or wsp
                fp8 = wname in FP8_W
                nb = PACKED_W[wname][0]
                for b_ in (range(nb) if blocks is None else blocks):
                    if b_ == 0 and w0 is not None:
                        wc = w0
                    else:
                        wc = pool.tile([P, src_kc, 256],
                                       FP8 if fp8 else BF16, tag="w")
                        nc.sync.dma_start(wc[:], wd[wname].ap()[b_])
                    if hook is not None:
                        hook(b_)
                    for ml in range(2):
                        c = 2 * b_ + ml
                        ps = psp.tile([P, T], F32, tag="mm")
                        mm_block(ps, wc, src, src_kc, ml, fp8)
                        b_sl = None if bias is None else bias[:, c:c + 1]
                        nc.scalar.activation(dst[:, c, :], ps[:], act,
                                             bias=b_sl,
                                             scale=(scale if scale is not None
                                                    else 1.0))

            def out_proj(ydram, wname, src, src_kc, bias, wpool,
                         evict_eng=None, w0=()):
                """y[:, c, :] = Wc.T @ src + bias_c -> DMA to DRAM (fp32).
                Drain via ACT Identity, eviction DMA issued from GpSimd (or
                the given engine — the final out_proj uses the idle Sync
                HWDGE so the tail eviction isn't behind the GpSimd DGE)."""
                evict_eng = evict_eng or nc.gpsimd
                fp8 = wname in FP8_W
                nb = PACKED_W[wname][0]
                for b_ in range(nb):
                    if b_ < len(w0):
                        wc = w0[b_]
                    else:
                        wc = wpool.tile([P, src_kc, 256],
                                        FP8 if fp8 else BF16, tag="w")
                        nc.sync.dma_start(wc[:], wd[wname].ap()[b_])
                    for ml in range(2):
                        c = 2 * b_ + ml
                        ps = psp.tile([P, T], F32, tag="mm")
                        mm_block(ps, wc, src, src_kc, ml, fp8)
                        yt = ypool.tile([P, T], F32, tag="y")
                        # drain on the ACT engine (idle during out_proj) so
                        # the psum-release rate keeps up with fp8 matmuls
                        sc_ = (bt[FP8_OUT_SCALE[wname]][:, 0:1] if fp8
                               else 1.0)
                        nc.scalar.activation(yt[:], ps[:], ACT.Identity,
                                             bias=bias[:, c:c + 1],
                                             scale=sc_)
                        evict_eng.dma_start(ydram.ap()[:, c, :], yt[:])

            # -- LayerNorm (g=1, b=0 verified on host): two-phase, PE-free --
            def ln_phase1(src, tag):
                """mu (bf16) and var+eps (f32), both [P,T] broadcast across
                partitions.  DVE chunk-chains + GpSimd partition reduce."""
                # ssum chain over chunks 0..6 then final add -> f32
                s_acc = None
                for c in range(KC - 2):
                    nxt = sqp.tile([P, T], BF16, tag="tr", name=f"s{tag}{c}")
                    if s_acc is None:
                        nc.vector.tensor_tensor(nxt[:], src[:, 0, :],
                                                src[:, 1, :], OP.add)
                    else:
                        nc.vector.tensor_tensor(nxt[:], s_acc[:],
                                                src[:, c + 1, :], OP.add)
                    s_acc = nxt
                s1f = lnsp.tile([P, T], F32, tag="st", bufs=3,
                                name=f"s1f{tag}")
                nc.vector.tensor_tensor(s1f[:], s_acc[:], src[:, KC - 1, :],
                                        OP.add)
                # ssq: square chunks then chain
                q_acc = None
                for c in range(KC):
                    sqc = sqp.tile([P, T], BF16, tag="sq", name=f"q{tag}{c}")
                    nc.vector.tensor_tensor(sqc[:], src[:, c, :],
                                            src[:, c, :], OP.mult)
                    if c == 1:
                        nxt = sqp.tile([P, T], BF16, tag="qp",
                                       name=f"qa{tag}{c}")
                        nc.vector.tensor_tensor(nxt[:], prev_sq[:], sqc[:],
                                                OP.add)
                        q_acc = nxt
                    elif c > 1 and c < KC - 1:
                        nxt = sqp.tile([P, T], BF16, tag="qp",
                                       name=f"qa{tag}{c}")
                        nc.vector.tensor_tensor(nxt[:], q_acc[:], sqc[:],
                                                OP.add)
                        q_acc = nxt
                    elif c == KC - 1:
                        q1f = lnsp.tile([P, T], F32, tag="st", bufs=3,
                                        name=f"q1f{tag}")
                        nc.vector.tensor_tensor(q1f[:], q_acc[:], sqc[:],
                                                OP.add)
                    prev_sq = sqc
                # cross-partition all-reduce (result broadcast to all parts)
                S_ = lnsp.tile([P, T], F32, tag="st", bufs=3, name=f"S{tag}")
                nc.gpsimd.partition_all_reduce(S_[:], s1f[:], P, RED.add)
                Q_ = lnsp.tile([P, T], F32, tag="st", bufs=3, name=f"Q{tag}")
                nc.gpsimd.partition_all_reduce(Q_[:], q1f[:], P, RED.add)
                # stats math (DVE, f32)
                mu_f = lnsp.tile([P, T], F32, tag="sc", bufs=2,
                                 name=f"muf{tag}")
                nc.vector.tensor_scalar(mu_f[:], S_[:], 1.0 / H, None,
                                        OP.mult)
                mu_b = lnsp.tile([P, T], BF16, tag="mb", bufs=1,
                                 name=f"mub{tag}")
                nc.vector.tensor_copy(mu_b[:], mu_f[:])
                s2 = lnsp.tile([P, T], F32, tag="sc", bufs=2, name=f"s2{tag}")
                nc.vector.tensor_tensor(s2[:], mu_f[:], mu_f[:], OP.mult)
                q1h = lnsp.tile([P, T], F32, tag="sc", bufs=2,
                                name=f"q1h{tag}")
                nc.vector.tensor_scalar(q1h[:], Q_[:], 1.0 / H, None, OP.mult)
                var_eps = lnsp.tile([P, T], F32, tag="ve", bufs=1,
                                    name=f"ve{tag}")
                nc.vector.scalar_tensor_tensor(var_eps[:], q1h[:], 1e-5,
                                               s2[:], OP.add, OP.subtract)
                return mu_b, var_eps

            def ln_phase2(dst, src, mu_b, var_eps, tag):
                """dst = (src - mu) * rsqrt(var+eps)  (bf16; rstd via ACT)."""
                sdev = lnsp.tile([P, T], F32, tag="sd", bufs=1,
                                 name=f"sd{tag}")
                nc.scalar.activation(sdev[:], var_eps[:], ACT.Sqrt)
                rs_b = lnsp.tile([P, T], BF16, tag="rb", bufs=1,
                                 name=f"rsb{tag}")
                with nc.allow_low_precision(reason="rstd in bf16 is ~0.1% "
                                            "rms; well inside tolerance"):
                    nc.vector.reciprocal(rs_b[:], sdev[:])
                for kc in range(KC):
                    t_ = sqp.tile([P, T], BF16, tag="tr", name=f"n{tag}{kc}")
                    nc.vector.tensor_tensor(t_[:], src[:, kc, :], mu_b[:],
                                            OP.subtract)
                    nc.vector.tensor_tensor(dst[:, kc, :], t_[:], rs_b[:],
                                            OP.mult)

            # ---- expert 1 first: its fp8 weights are half the startup ----
            # bytes and the DoubleRow pace matches the cold DMA ring, so
            # the PE never starves while the rest of the model streams in
            xt1 = xpool.tile([P, KC, T], FP8, tag="xg", name="xgt1")
            wc10 = wsp.tile([P, KC, 256], FP8, tag="w", name="wc10")
            nc.sync.dma_start(xt1[:, 0:4, :], xg[1].ap()[:, 0:4, :])
            nc.scalar.dma_start(wc10[:], wd["c1p"].ap()[0])
            nc.sync.dma_start(xt1[:, 4:8, :], xg[1].ap()[:, 4:8, :])

            # prefetch tiles with their own tags so the ring-slot reuse of
            # the main weight pools isn't disturbed
            wc20 = wsp2.tile([P, K2, 256], FP8, tag="wc2", bufs=2,
                             name="wc20")
            wc21 = wsp2.tile([P, K2, 256], FP8, tag="wc2", bufs=2,
                             name="wc21")
            wa0e = wsp.tile([P, KC, 256], BF16, tag="w0e", bufs=2,
                            name="wa0e")
            wb0e = wsp.tile([P, KC, 256], BF16, tag="w0e", bufs=2,
                            name="wb0e")

            def c1p_hook(b_):
                # pull the next phases' first weight blocks ahead in the
                # sync DMA queue so the c2p / E0 transitions don't starve
                if b_ == 3:
                    nc.sync.dma_start(wc20[:], wd["c2p"].ap()[0])
                elif b_ == 5:
                    nc.sync.dma_start(wc21[:], wd["c2p"].ap()[1])
                elif b_ == 6:
                    nc.sync.dma_start(wa0e[:], wd["w1p"].ap()[0])
                elif b_ == 7:
                    nc.sync.dma_start(wb0e[:], wd["w3p"].ap()[0])

            g1 = ipool.tile([P, K2, T], FP8, tag="tD", name="g1")
            up_proj(g1, "c1p", xt1, KC, ACT.Gelu, bt["c1b"],
                    scale=bt["c1s"][:, 0:1], w0=wc10, hook=c1p_hook)
            out_proj(ys[1], "c2p", g1, K2, bt["c2b"], wsp2,
                     w0=[wc20, wc21])

            # ---- expert 0 up-proj, with E2's folded front (a2p) tucked ---
            # between its last blocks so a2p's bf16 weights load outside
            # the ring-tight startup window
            xt0 = load_xg(0)

            def e0_blocks(blocks):
                for b_ in blocks:
                    if b_ == 0:
                        wa, wb = wa0e, wb0e
                    else:
                        wa = wsp.tile([P, KC, 256], BF16, tag="w")
                        nc.sync.dma_start(wa[:], wd["w1p"].ap()[b_])
                        wb = wsp.tile([P, KC, 256], BF16, tag="w")
                        nc.sync.dma_start(wb[:], wd["w3p"].ap()[b_])
                    for ml in range(2):
                        c = 2 * b_ + ml
                        psa = psp.tile([P, T], F32, tag="mm")
                        psb = psp.tile([P, T], F32, tag="mm")
                        for kc in range(KC):
                            nc.tensor.matmul(
                                psa[:], wa[:, kc, ml * P:(ml + 1) * P],
                                xt0[:, kc, :],
                                start=(kc == 0), stop=(kc == KC - 1))
                        for kc in range(KC):
                            nc.tensor.matmul(
                                psb[:], wb[:, kc, ml * P:(ml + 1) * P],
                                xt0[:, kc, :],
                                start=(kc == 0), stop=(kc == KC - 1))
                        sa = ypool.tile([P, T], BF16, tag="sa")
                        nc.scalar.activation(sa[:], psa[:], ACT.Silu)
                        nc.vector.scalar_tensor_tensor(
                            h1f[:, c, :], psb[:], bt["hsc"][:, 0:1], sa[:],
                            OP.mult, OP.mult)

            e0_blocks(range(0, 12))

            # ---- expert 2 (part 1): folded front + LN1 stats -------------
            xt2 = load_xg(2)
            t2 = ipool.tile([P, KC, T], BF16, tag="tA", name="t2")
            up_proj(t2, "a2p", xt2, KC, ACT.Identity, bt["a2b"])
            mu1, ve1 = ln_phase1(t2, "l1")

            e0_blocks(range(12, 16))
            h2 = ipool.tile([P, KC, T], BF16, tag="tC", name="h2")
            ln_phase2(h2, t2, mu1, ve1, "l1")
            out_proj(ys[0], "w2p", h1f, KI, bt["zb"], wbp)

            # ---- expert 2 (part 2): FF + residual + LN2 stats ------------
            g2 = ipool.tile([P, K2, T], BF16, tag="tD", name="g2")
            up_proj(g2, "f1p", h2, KC, ACT.Relu, bt["f1b"])
            ffa = ipool.tile([P, KC, T], BF16, tag="tB", name="ffa")
            nb_f2 = PACKED_W["f2p"][0]
            for b_ in range(nb_f2):
                wc = wsp2.tile([P, K2, 256], BF16, tag="w")
                nc.sync.dma_start(wc[:], wd["f2p"].ap()[b_])
                for ml in range(2):
                    c = 2 * b_ + ml
                    ps = psp.tile([P, T], F32, tag="mm")
                    for kc in range(K2):
                        nc.tensor.matmul(ps[:], wc[:, kc, ml * P:(ml + 1) * P],
                                         g2[:, kc, :],
                                         start=(kc == 0), stop=(kc == K2 - 1))
                    # ffa = ff + f2b + h2   (residual)
                    nc.vector.scalar_tensor_tensor(
                        ffa[:, c, :], ps[:], bt["f2b"][:, c:c + 1],
                        h2[:, c, :], OP.add, OP.add)
            # ---- expert 3 up-proj; LN2 math hidden underneath ------------
            xt3 = load_xg(3)
            mu2, ve2 = ln_phase1(ffa, "l2")
            h1 = hpool.tile([P, KI, T], BF16, tag="h1", name="h1e3")
            up_proj(h1, "m1p", xt3, KC, ACT.Gelu, bt["m1b"],
                    blocks=range(0, 8))
            h2b = ipool.tile([P, KC, T], BF16, tag="tA", name="h2b")
            ln_phase2(h2b, ffa, mu2, ve2, "l2")
            up_proj(h1, "m1p", xt3, KC, ACT.Gelu, bt["m1b"],
                    blocks=range(8, 16))
            out_proj(ys[2], "genp", h2b, KC, bt["genb"], wsp)

            # ---- expert 3 down-projection --------------------------------
            out_proj(ys[3], "m2p", h1, KI, bt["m2b"], wbp, evict_eng=nc.sync)

    nc.compile()
    return nc


_PROGRAM = None


def _get_program():
    global _PROGRAM
    if _PROGRAM is None:
        _PROGRAM = build_moe_sparse()
    return _PROGRAM


def run_cores(nc, in_maps, trace=False, trace_cores=None):
    if trace:
        _install_ntff_shim()
    return run_bass_kernel_spmd(nc, in_maps, core_ids=list(range(len(in_maps))),
                                trace=trace, trace_cores=trace_cores)


# ---- host side ---------------------------------------------------------
def _gelu(x):
    try:
        from scipy.special import erf
        return 0.5 * x * (1.0 + erf(x / math.sqrt(2.0)))
    except ImportError:
        ve = np.vectorize(math.erf)
        return 0.5 * x * (1.0 + ve(x / math.sqrt(2.0)))


def _ln64(h, g, b, eps=1e-5):
    mu = h.mean(-1, keepdims=True)
    var = ((h - mu) ** 2).mean(-1, keepdims=True)
    return (h - mu) / np.sqrt(var + eps) * g + b


def _pack_w(w, kcc, dt=None):
    """[K, M] fp64 -> [M//256, P, kcc, 256] contiguous tile blocks."""
    K, M = w.shape
    assert K == kcc * P
    r = w.reshape(kcc, P, M)
    blocks = [np.ascontiguousarray(r[:, :, b * 256:(b + 1) * 256]
                                   .transpose(1, 0, 2))
              for b in range(M // 256)]
    return np.stack(blocks, 0).astype(dt or BF)


def _p2scale(a, target=96.0):
    """Power-of-2 scale putting |a|max near `target` (e4m3 max is 240)."""
    am = float(np.abs(a).max())
    if am == 0.0:
        return 1.0
    return 2.0 ** round(math.log2(target / am))


def _pack_b(b):
    n = b.shape[0] // P
    return np.ascontiguousarray(b.reshape(n, P).T.astype(np.float32))


def prepare(inputs):
    f64 = lambda n: np.asarray(inputs[n], np.float64)
    x = np.asarray(inputs["x"], np.float32).reshape(-1, H)

    # the device LN path hardcodes gamma=1, beta=0 (the reference always
    # passes ones/zeros); verify that assumption on the actual inputs
    assert np.allclose(np.asarray(inputs["ce_n1g"]), 1.0), "ce_n1g != 1"
    assert np.allclose(np.asarray(inputs["ce_n1b"]), 0.0), "ce_n1b != 0"
    assert np.allclose(np.asarray(inputs["ce_n2g"]), 1.0), "ce_n2g != 1"
    assert np.allclose(np.asarray(inputs["ce_n2b"]), 0.0), "ce_n2b != 0"

    # routing (host, fp64)
    lg = x.astype(np.float64) @ f64("router_w")
    lg += f64("router_b") + f64("load_balancer")
    sel = np.argsort(-lg, axis=1, kind="stable")[:, :2]
    ls = np.take_along_axis(lg, sel, 1)
    ew = np.exp(ls - ls.max(1, keepdims=True))
    gates = ew / ew.sum(1, keepdims=True)

    # folded weights (fp64)
    F = {}
    F["A1"] = f64("me_eq_w") @ f64("me_wv") @ f64("me_wo")
    F["a1"] = (f64("me_eq_b") @ f64("me_wv") + f64("me_bv")) @ f64("me_wo") \
        + f64("me_bo")
    W2o = f64("ce_wv") @ f64("ce_wo")
    F["A2"] = f64("ce_syn_w") + f64("ce_syn_w") @ W2o
    F["a2"] = f64("ce_syn_b") + f64("ce_syn_b") @ W2o + f64("ce_bv") \
        @ f64("ce_wo") + f64("ce_bo")

    C1 = F["A1"] @ f64("me_c1w")
    # fp8 scaling (power-of-2; weight absmax exact, activation absmax
    # estimated on a token subsample with a 4x saturation margin)
    sx1 = _p2scale(x)
    sc1 = _p2scale(C1)
    sc2 = _p2scale(f64("me_c2w"))
    xs = x[::16].astype(np.float32)
    a_ = xs @ np.asarray(inputs["sw_w1"], np.float32)
    h1s = a_ / (1.0 + np.exp(-a_)) * (xs @ np.asarray(inputs["sw_w3"],
                                                     np.float32))
    sh = _p2scale(h1s, target=48.0)
    sw2p = _p2scale(f64("sw_w2"))
    wmap = {
        "w1p": (f64("sw_w1"), KC), "w3p": (f64("sw_w3"), KC),
        "w2p": (f64("sw_w2") * sw2p, KI),
        "c1p": (C1 * sc1, KC),
        "c2p": (f64("me_c2w") * sc2, K2),
        "a2p": (F["A2"], KC), "f1p": (f64("ce_f1w"), KC),
        "f2p": (f64("ce_f2w"), K2), "genp": (f64("ce_gen_w"), KC),
        "m1p": (f64("ml_w1"), KC), "m2p": (f64("ml_w2"), KI),
    }
    bmap = {
        "c1b": F["a1"] @ f64("me_c1w") + f64("me_c1b"),
        "c2b": f64("me_c2b"),
        "a2b": F["a2"],
        "f1b": f64("ce_f1b"), "f2b": f64("ce_f2b"),
        "n1g": f64("ce_n1g"), "n1b": f64("ce_n1b"),
        "n2g": f64("ce_n2g"), "n2b": f64("ce_n2b"),
        "genb": f64("ce_gen_b"), "m1b": f64("ml_b1"), "m2b": f64("ml_b2"),
        "zb": np.zeros(H),
        "c1s": np.full(P, 1.0 / (sx1 * sc1)),
        "c2s": np.full(P, 1.0 / sc2),
        "w2s": np.full(P, 1.0 / (sh * sw2p)),
        "hsc": np.full(P, sh),
    }
    base = {n: _pack_w(w, kcc, F8 if n in FP8_W else None)
            for n, (w, kcc) in wmap.items()}
    base["ball"] = np.concatenate([_pack_b(bmap[n]) for n in BIASES], 1)

    meta = {"x": x, "gates": gates, "sel": sel, "F": F,
            "dev_idx": [], "dev_w": [], "ovf": []}
    in_maps = [dict(base) for _ in range(N_CORES)]
    for e in range(E):
        m = sel == e
        tok = np.nonzero(m.any(1))[0]
        we = np.where(m[:, 0][tok], gates[tok, 0], gates[tok, 1])
        dev, ovf = tok[:CAP], tok[CAP:]
        meta["dev_idx"].append(dev)
        meta["dev_w"].append(we[:len(dev)])
        meta["ovf"].append((ovf, we[len(dev):]))
        xfull = np.zeros((CAP, H), np.float32)
        xfull[:len(dev)] = x[dev]
        percore = xfull.reshape(N_CORES, T, H)
        for c in range(N_CORES):
            xc = percore[c].T.reshape(KC, P, T).transpose(1, 0, 2)
            if e == 1:
                in_maps[c][f"xg{e}"] = np.ascontiguousarray(
                    xc * sx1).astype(F8)
            else:
                in_maps[c][f"xg{e}"] = np.ascontiguousarray(xc).astype(BF)
    meta["in_maps"] = in_maps
    return meta


def _host_expert(e, xs, inputs, F):
    """Overflow tokens, fp64, replicating the reference formulas."""
    f64 = lambda n: np.asarray(inputs[n], np.float64)
    xs = xs.astype(np.float64)
    if e == 0:
        a = xs @ f64("sw_w1")
        g = a / (1.0 + np.exp(-a)) * (xs @ f64("sw_w3"))
        return g @ f64("sw_w2")
    if e == 1:
        t = xs @ F["A1"] + F["a1"]
        g = _gelu(t @ f64("me_c1w") + f64("me_c1b"))
        return g @ f64("me_c2w") + f64("me_c2b")
    if e == 2:
        t = xs @ F["A2"] + F["a2"]
        h2 = _ln64(t, f64("ce_n1g"), f64("ce_n1b"))
        ff = np.maximum(h2 @ f64("ce_f1w") + f64("ce_f1b"), 0.0) \
            @ f64("ce_f2w") + f64("ce_f2b")
        h2 = _ln64(h2 + ff, f64("ce_n2g"), f64("ce_n2b"))
        return h2 @ f64("ce_gen_w") + f64("ce_gen_b")
    a = _gelu(xs @ f64("ml_w1") + f64("ml_b1"))
    return a @ f64("ml_w2") + f64("ml_b2")


def combine(meta, results, inputs):
    out = np.zeros((B * S, H), np.float32)
    for e in range(E):
        ye = np.concatenate(
            [results[c][f"y{e}"].transpose(2, 1, 0).reshape(T, H)
             for c in range(N_CORES)], 0)
        dev, we = meta["dev_idx"][e], meta["dev_w"][e]
        out[dev] += (we[:, None] * ye[:len(dev)]).astype(np.float32)
        ovf, wo = meta["ovf"][e]
        if len(ovf):
            yh = _host_expert(e, meta["x"][ovf], inputs, meta["F"])
            out[ovf] += (wo[:, None] * yh).astype(np.float32)
    return out.reshape(B, S, H)


def kernel(**inputs):
    nc = _get_program()
    meta = prepare(inputs)
    # transient NRT/axon device errors (UNAVAILABLE / INTERNAL) have been
    # observed on this fleet and clear on re-run: retry a bounded number
    # of times rather than failing the whole call
    last = None
    for _ in range(3):
        try:
            res = run_cores(nc, meta["in_maps"])
            break
        except Exception as e:
            last = e
    else:
        raise last
    return combine(meta, [res.results[c] for c in range(N_CORES)], inputs)


# ---- NTFF profiling shim (axon) — used by test.py only ----------------
def _install_ntff_shim():
    import contextlib
    import ctypes
    import sys
    import types

    if "antenv.axon_hooks" in sys.modules:
        return
    lib = ctypes.CDLL("/opt/axon/libaxon_pjrt.so")
    if not hasattr(lib, "axon_start_nrt_profile"):
        return
    lib.axon_start_nrt_profile.argtypes = [ctypes.POINTER(ctypes.c_int64),
                                           ctypes.c_size_t]
    lib.axon_start_nrt_profile.restype = ctypes.c_int64
    lib.axon_stop_nrt_profile.argtypes = [ctypes.c_char_p]
    lib.axon_stop_nrt_profile.restype = ctypes.c_int64

    @contextlib.contextmanager
    def _hook(output_dir, device_ids):
        import jax
        jax.devices()
        if device_ids:
            ids = (ctypes.c_int64 * len(device_ids))(*device_ids)
            rc = lib.axon_start_nrt_profile(ids, len(device_ids))
        else:
            rc = lib.axon_start_nrt_profile(None, 0)
        if rc != 0:
            raise RuntimeError(f"axon_start_nrt_profile rc={rc}")
        try:
            yield
        finally:
            n = lib.axon_stop_nrt_profile(str(output_dir).encode())
            print(f"profile: {n} file(s) written to {output_dir}",
                  file=sys.stderr)

    import antenv
    mod = types.ModuleType("antenv.axon_hooks")
    mod.get_axon_ntff_profile_hook = lambda: _hook
    mod.set_axon_ntff_profile_hook = lambda hk: None
    sys.modules["antenv.axon_hooks"] = mod
    antenv.axon_hooks = mod


# revision 47
# speedup vs baseline: 1.1881x; 1.1881x over previous
"""Trainium2 Bass kernel for nn_MixtureOfExpertsLayer (moe_routing).

Sparse dispatch: top-2 routing is computed on the host (the router is a
tiny [8192,1024]@[1024,4] GEMM); tokens are gathered per expert and
sharded across the 8 cores so each core runs a fixed 512-token slab
through each of the 4 experts — half the dense FLOPs.  The linear
chains inside experts 1/2 are pre-folded on the host
(eq_w@wv@wo and syn_w@(I+wv@wo)), removing another ~11% of matmul work.

Device compute is bf16 (PSUM accumulates fp32).  Weights are pre-packed
on the host into the exact [p, kc, 256-col] tile layout the tensor
engine wants, so every DMA is a fully contiguous 0.5-2MB block.
Activations stay feature-major [128p, chunk, tok]; every matmul has a
512-token moving dim (full PE rate).  Expert outputs come back
feature-major [H, 512] fp32; the host applies the top-2 softmax gates
and scatter-adds into the final output.  Tokens beyond the
4096-per-expert device capacity (a few dozen when routing is balanced)
are computed on the host in fp64.

PE-roofline tuning (vs the first working version):
 - LayerNorm stats no longer use PE matmuls: per-chunk sums/squares are
   tree-summed on the DVE and reduced across partitions with the
   GpSimd daisy-chain partition_all_reduce (result already broadcast),
   freeing ~8us of tensor-engine time and two PSUM pools.
 - The first E0 weight/activation DMAs are split into quarter-chunks so
   the first matmul starts as soon as ~0.4MB (not 2MB) has landed, and
   a short burst of dummy matmuls warms the PE HAM clock-gate during
   the initial DMA fill (cold PE runs at 1.2GHz, warm at 2.4GHz).
 - Output-eviction DMAs are issued from the (otherwise idle) GpSimd
   DGE so weight-prefetch issue on the Sync engine is never queued
   behind them; the matmul PSUM pool gets 6 banks.
"""
import math

import numpy as np
import ml_dtypes

import concourse.bass as bass
import concourse.mybir as mybir
import concourse.tile as tile
from concourse import bacc
from concourse.alu_op_type import AluOpType
from concourse.bass_utils import run_bass_kernel_spmd

F32 = mybir.dt.float32
BF16 = mybir.dt.bfloat16
FP8 = mybir.dt.float8e4
ACT = mybir.ActivationFunctionType
AX = mybir.AxisListType
OP = AluOpType
RED = bass.bass_isa.ReduceOp
DR = mybir.MatmulPerfMode.DoubleRow
BF = ml_dtypes.bfloat16
F8 = ml_dtypes.float8_e4m3

N_CORES = 8
B, S, H, I, E = 4, 2048, 1024, 4096, 4
P = 128
T = 512                   # tokens per expert per core
CAP = N_CORES * T         # device capacity per expert
KC = H // P               # 8
KI = I // P               # 32
K2 = (2 * H) // P         # 16

# packed weight dram tensors: name -> (n_256col_blocks, contraction_chunks)
PACKED_W = {
    "w1p": (I // 256, KC), "w3p": (I // 256, KC), "m1p": (I // 256, KC),
    "w2p": (H // 256, KI), "m2p": (H // 256, KI),
    "c1p": (2 * H // 256, KC), "f1p": (2 * H // 256, KC),
    "c2p": (H // 256, K2), "f2p": (H // 256, K2),
    "a2p": (H // 256, KC), "genp": (H // 256, KC),
}
# expert 1 and E0's down-projection run in fp8-e4m3 with DoubleRow
# (simulated total error 0.0147 vs the 2e-2 gate); weights are
# pre-scaled by a power of two on the host and the inverse scales ride
# in the bias pack
FP8_W = {"c1p", "c2p", "w2p"}
FP8_OUT_SCALE = {"c2p": "c2s", "w2p": "w2s"}
# biases live in one packed [P, sum] f32 tensor; name -> n_chunks
BIASES = {
    "c1b": K2, "c2b": KC,
    "a2b": KC, "f1b": K2, "f2b": KC,
    "n1g": KC, "n1b": KC, "n2g": KC, "n2b": KC, "genb": KC,
    "m1b": KI, "m2b": KC, "zb": KC,
    "c1s": 1, "c2s": 1, "w2s": 1, "hsc": 1,
}
BIAS_OFF = {}
_off = 0
for _n, _c in BIASES.items():
    BIAS_OFF[_n] = _off
    _off += _c
BIAS_COLS = _off


def build_moe_sparse():
    nc = bacc.Bacc("TRN2", target_bir_lowering=False, debug=False)

    xg = [nc.dram_tensor(f"xg{e}", [P, KC, T], FP8 if e == 1 else BF16,
                         kind="ExternalInput")
          for e in range(E)]
    wd = {n: nc.dram_tensor(n, [nb, P, kcc, 256],
                            FP8 if n in FP8_W else BF16,
                            kind="ExternalInput")
          for n, (nb, kcc) in PACKED_W.items()}
    ball = nc.dram_tensor("ball", [P, BIAS_COLS], F32, kind="ExternalInput")
    ys = [nc.dram_tensor(f"y{e}", [P, KC, T], F32, kind="ExternalOutput")
          for e in range(E)]

    with tile.TileContext(nc) as tc:
        with (
            tc.tile_pool(name="const", bufs=1) as cpool,
            tc.tile_pool(name="xg", bufs=2) as xpool,
            tc.tile_pool(name="h1", bufs=1) as hpool,
            tc.tile_pool(name="inter", bufs=1) as ipool,
            tc.tile_pool(name="ws", bufs=5) as wsp,     # KC-contraction blocks
            tc.tile_pool(name="ws2", bufs=3) as wsp2,   # K2-contraction blocks
            tc.tile_pool(name="wb", bufs=2) as wbp,     # KI-contraction blocks
            tc.tile_pool(name="yev", bufs=3) as ypool,  # output eviction
            tc.tile_pool(name="lns", bufs=2) as lnsp,   # LN stat tiles
            tc.tile_pool(name="sq", bufs=2) as sqp,     # LN scratch (bf16)
            tc.tile_pool(name="ps", bufs=7, space=bass.MemorySpace.PSUM) as psp,
            tc.tile_pool(name="psw", bufs=1, space=bass.MemorySpace.PSUM) as pswp,
        ):
            # ---- constants ------------------------------------------------
            ones_c = cpool.tile([P, 1], BF16, tag="ones_c")
            nc.vector.memset(ones_c[:], 1.0)
            dmt = cpool.tile([P, T], BF16, tag="dmt")
            nc.vector.memset(dmt[:], 0.0078125)

            bt_all = cpool.tile([P, BIAS_COLS], F32, tag="ball")
            bt = {n: bt_all[:, BIAS_OFF[n]:BIAS_OFF[n] + nch]
                  for n, nch in BIASES.items()}

            # ---- HAM warm-up: dummy matmuls fill the initial DMA wait ----
            wps = pswp.tile([1, T], F32, tag="warm")

            def warm(n):
                # dummy matmuls on resident constants: keep the PE HAM
                # activity window fed while the startup DMAs stream in
                for _ in range(n):
                    nc.tensor.matmul(wps[:], ones_c[:], dmt[:],
                                     start=True, stop=True)

            # a short contiguous burst while the first DMAs stream in; the
            # startup is ring-throughput-bound, so the early real matmuls
            # run at the cold-clock pace that matches the ring anyway
            warm(7)

            # E0's intermediate is written directly in scaled fp8 for the
            # w2p DoubleRow down-projection; E3 later reuses the same pool
            # slot for its bf16 intermediate (disjoint lifetimes)
            h1f = hpool.tile([P, KI, T], FP8, tag="h1", name="h1f")
            # bias/scale pack: tiny (36KB) — load first on the idle gpsimd
            # DGE so the fp8 scale columns are resident before E0's first
            # h1 write
            nc.gpsimd.dma_start(bt_all[:], ball.ap())

            # ---- helpers --------------------------------------------------
            def load_xg(e, split=1):
                t_ = xpool.tile([P, KC, T], FP8 if e == 1 else BF16,
                                tag="xg", name=f"xgt{e}")
                step = KC // split
                for i in range(split):
                    sl = slice(i * step, (i + 1) * step)
                    nc.sync.dma_start(t_[:, sl, :], xg[e].ap()[:, sl, :])
                return t_

            def mm_block(ps, wc, src, src_kc, ml, fp8):
                """Accumulate one 256-col block into ps; fp8 runs DoubleRow
                (two contraction chunks per instruction at 2x rate)."""
                if fp8:
                    for kc in range(0, src_kc, 2):
                        nc.tensor.matmul(
                            ps[:], wc[:, kc:kc + 2, ml * P:(ml + 1) * P],
                            src[:, kc:kc + 2, :],
                            start=(kc == 0), stop=(kc == src_kc - 2),
                            perf_mode=DR)
                else:
                    for kc in range(src_kc):
                        nc.tensor.matmul(
                            ps[:], wc[:, kc, ml * P:(ml + 1) * P],
                            src[:, kc, :],
                            start=(kc == 0), stop=(kc == src_kc - 1))

            def up_proj(dst, wname, src, src_kc, act, bias, blocks=None,
                        pool=None, scale=None, w0=None, hook=None):
                """dst[:, c, :] = act(scale * (Wc.T @ src) + bias_c),
                streamed in 256-col blocks.  dst chunk c = 2*b + ml."""
                pool = pool or wsp
                fp8 = wname in FP8_W
                nb = PACKED_W[wname][0]
                for b_ in (range(nb) if blocks is None else blocks):
                    if b_ == 0 and w0 is not None:
                        wc = w0
                    else:
                        wc = pool.tile([P, src_kc, 256],
                                       FP8 if fp8 else BF16, tag="w")
                        nc.sync.dma_start(wc[:], wd[wname].ap()[b_])
                    if hook is not None:
                        hook(b_)
                    for ml in range(2):
                        c = 2 * b_ + ml
                        ps = psp.tile([P, T], F32, tag="mm")
                        mm_block(ps, wc, src, src_kc, ml, fp8)
                        b_sl = None if bias is None else bias[:, c:c + 1]
                        nc.scalar.activation(dst[:, c, :], ps[:], act,
                                             bias=b_sl,
                                             scale=(scale if scale is not None
                                                    else 1.0))

            def out_proj(ydram, wname, src, src_kc, bias, wpool,
                         evict_eng=None, w0=()):
                """y[:, c, :] = Wc.T @ src + bias_c -> DMA to DRAM (fp32).
                Drain via ACT Identity, eviction DMA issued from GpSimd (or
                the given engine — the final out_proj uses the idle Sync
                HWDGE so the tail eviction isn't behind the GpSimd DGE)."""
                evict_eng = evict_eng or nc.gpsimd
                fp8 = wname in FP8_W
                nb = PACKED_W[wname][0]
                for b_ in range(nb):
                    if b_ < len(w0):
                        wc = w0[b_]
                    else:
                        wc = wpool.tile([P, src_kc, 256],
                                        FP8 if fp8 else BF16, tag="w")
                        nc.sync.dma_start(wc[:], wd[wname].ap()[b_])
                    for ml in range(2):
                        c = 2 * b_ + ml
                        ps = psp.tile([P, T], F32, tag="mm")
                        mm_block(ps, wc, src, src_kc, ml, fp8)
                        yt = ypool.tile([P, T], F32, tag="y")
                        # drain on the ACT engine (idle during out_proj) so
                        # the psum-release rate keeps up with fp8 matmuls
                        sc_ = (bt[FP8_OUT_SCALE[wname]][:, 0:1] if fp8
                               else 1.0)
                        nc.scalar.activation(yt[:], ps[:], ACT.Identity,
                                             bias=bias[:, c:c + 1],
                                             scale=sc_)
                        evict_eng.dma_start(ydram.ap()[:, c, :], yt[:])

            # -- LayerNorm (g=1, b=0 verified on host): two-phase, PE-free --
            def ln_phase1(src, tag):
                """mu (bf16) and var+eps (f32), both [P,T] broadcast across
                partitions.  DVE chunk-chains + GpSimd partition reduce."""
                # ssum chain over chunks 0..6 then final add -> f32
                s_acc = None
                for c in range(KC - 2):
                    nxt = sqp.tile([P, T], BF16, tag="tr", name=f"s{tag}{c}")
                    if s_acc is None:
                        nc.vector.tensor_tensor(nxt[:], src[:, 0, :],
                                                src[:, 1, :], OP.add)
                    else:
                        nc.vector.tensor_tensor(nxt[:], s_acc[:],
                                                src[:, c + 1, :], OP.add)
                    s_acc = nxt
                s1f = lnsp.tile([P, T], F32, tag="st", bufs=3,
                                name=f"s1f{tag}")
                nc.vector.tensor_tensor(s1f[:], s_acc[:], src[:, KC - 1, :],
                                        OP.add)
                # ssq: square chunks then chain
                q_acc = None
                for c in range(KC):
                    sqc = sqp.tile([P, T], BF16, tag="sq", name=f"q{tag}{c}")
                    nc.vector.tensor_tensor(sqc[:], src[:, c, :],
                                            src[:, c, :], OP.mult)
                    if c == 1:
                        nxt = sqp.tile([P, T], BF16, tag="qp",
                                       name=f"qa{tag}{c}")
                        nc.vector.tensor_tensor(nxt[:], prev_sq[:], sqc[:],
                                                OP.add)
                        q_acc = nxt
                    elif c > 1 and c < KC - 1:
                        nxt = sqp.tile([P, T], BF16, tag="qp",
                                       name=f"qa{tag}{c}")
                        nc.vector.tensor_tensor(nxt[:], q_acc[:], sqc[:],
                                                OP.add)
                        q_acc = nxt
                    elif c == KC - 1:
                        q1f = lnsp.tile([P, T], F32, tag="st", bufs=3,
                                        name=f"q1f{tag}")
                        nc.vector.tensor_tensor(q1f[:], q_acc[:], sqc[:],
                                                OP.add)
                    prev_sq = sqc
                # cross-partition all-reduce (result broadcast to all parts)
                S_ = lnsp.tile([P, T], F32, tag="st", bufs=3, name=f"S{tag}")
                nc.gpsimd.partition_all_reduce(S_[:], s1f[:], P, RED.add)
                Q_ = lnsp.tile([P, T], F32, tag="st", bufs=3, name=f"Q{tag}")
                nc.gpsimd.partition_all_reduce(Q_[:], q1f[:], P, RED.add)
                # stats math (DVE, f32)
                mu_f = lnsp.tile([P, T], F32, tag="sc", bufs=2,
                                 name=f"muf{tag}")
                nc.vector.tensor_scalar(mu_f[:], S_[:], 1.0 / H, None,
                                        OP.mult)
                mu_b = lnsp.tile([P, T], BF16, tag="mb", bufs=1,
                                 name=f"mub{tag}")
                nc.vector.tensor_copy(mu_b[:], mu_f[:])
                s2 = lnsp.tile([P, T], F32, tag="sc", bufs=2, name=f"s2{tag}")
                nc.vector.tensor_tensor(s2[:], mu_f[:], mu_f[:], OP.mult)
                q1h = lnsp.tile([P, T], F32, tag="sc", bufs=2,
                                name=f"q1h{tag}")
                nc.vector.tensor_scalar(q1h[:], Q_[:], 1.0 / H, None, OP.mult)
                var_eps = lnsp.tile([P, T], F32, tag="ve", bufs=1,
                                    name=f"ve{tag}")
                nc.vector.scalar_tensor_tensor(var_eps[:], q1h[:], 1e-5,
                                               s2[:], OP.add, OP.subtract)
                return mu_b, var_eps

            def ln_phase2(dst, src, mu_b, var_eps, tag):
                """dst = (src - mu) * rsqrt(var+eps)  (bf16; rstd via ACT)."""
                sdev = lnsp.tile([P, T], F32, tag="sd", bufs=1,
                                 name=f"sd{tag}")
                nc.scalar.activation(sdev[:], var_eps[:], ACT.Sqrt)
                rs_b = lnsp.tile([P, T], BF16, tag="rb", bufs=1,
                                 name=f"rsb{tag}")
                with nc.allow_low_precision(reason="rstd in bf16 is ~0.1% "
                                            "rms; well inside tolerance"):
                    nc.vector.reciprocal(rs_b[:], sdev[:])
                for kc in range(KC):
                    t_ = sqp.tile([P, T], BF16, tag="tr", name=f"n{tag}{kc}")
                    nc.vector.tensor_tensor(t_[:], src[:, kc, :], mu_b[:],
                                            OP.subtract)
                    nc.vector.tensor_tensor(dst[:, kc, :], t_[:], rs_b[:],
                                            OP.mult)

            # ---- expert 1 first: its fp8 weights are half the startup ----
            # bytes and the DoubleRow pace matches the cold DMA ring, so
            # the PE never starves while the rest of the model streams in
            xt1 = xpool.tile([P, KC, T], FP8, tag="xg", name="xgt1")
            wc10 = wsp.tile([P, KC, 256], FP8, tag="w", name="wc10")
            nc.sync.dma_start(xt1[:, 0:4, :], xg[1].ap()[:, 0:4, :])
            nc.scalar.dma_start(wc10[:], wd["c1p"].ap()[0])
            nc.sync.dma_start(xt1[:, 4:8, :], xg[1].ap()[:, 4:8, :])

            # prefetch tiles with their own tags so the ring-slot reuse of
            # the main weight pools isn't disturbed
            wc20 = wsp2.tile([P, K2, 256], FP8, tag="wc2", bufs=1,
                             name="wc20")
            wa0e = wsp.tile([P, KC, 256], BF16, tag="w0e", bufs=1,
                            name="wa0e")

            def c1p_hook(b_):
                # pull the next phases' first weight blocks ahead in the
                # sync DMA queue so the c2p / E0 transitions don't starve
                if b_ == 3:
                    nc.sync.dma_start(wc20[:], wd["c2p"].ap()[0])
                elif b_ == 6:
                    nc.sync.dma_start(wa0e[:], wd["w1p"].ap()[0])

            g1 = ipool.tile([P, K2, T], FP8, tag="tD", name="g1")
            up_proj(g1, "c1p", xt1, KC, ACT.Gelu, bt["c1b"],
                    scale=bt["c1s"][:, 0:1], w0=wc10, hook=c1p_hook)
            out_proj(ys[1], "c2p", g1, K2, bt["c2b"], wsp2,
                     w0=[wc20])

            # ---- expert 0 up-proj, with E2's folded front (a2p) tucked ---
            # between its last blocks so a2p's bf16 weights load outside
            # the ring-tight startup window
            xt0 = load_xg(0)

            def e0_blocks(blocks):
                for b_ in blocks:
                    if b_ == 0:
                        wa = wa0e
                    else:
                        wa = wsp.tile([P, KC, 256], BF16, tag="w")
                        nc.sync.dma_start(wa[:], wd["w1p"].ap()[b_])
                    wb = wsp.tile([P, KC, 256], BF16, tag="w")
                    nc.sync.dma_start(wb[:], wd["w3p"].ap()[b_])
                    for ml in range(2):
                        c = 2 * b_ + ml
                        psa = psp.tile([P, T], F32, tag="mm")
                        psb = psp.tile([P, T], F32, tag="mm")
                        for kc in range(KC):
                            nc.tensor.matmul(
                                psa[:], wa[:, kc, ml * P:(ml + 1) * P],
                                xt0[:, kc, :],
                                start=(kc == 0), stop=(kc == KC - 1))
                        for kc in range(KC):
                            nc.tensor.matmul(
                                psb[:], wb[:, kc, ml * P:(ml + 1) * P],
                                xt0[:, kc, :],
                                start=(kc == 0), stop=(kc == KC - 1))
                        sa = ypool.tile([P, T], BF16, tag="sa")
                        nc.scalar.activation(sa[:], psa[:], ACT.Silu)
                        nc.vector.scalar_tensor_tensor(
                            h1f[:, c, :], psb[:], bt["hsc"][:, 0:1], sa[:],
                            OP.mult, OP.mult)

            e0_blocks(range(0, 12))

            # ---- expert 2 (part 1): folded front + LN1 stats -------------
            xt2 = load_xg(2)
            t2 = ipool.tile([P, KC, T], BF16, tag="tA", name="t2")
            up_proj(t2, "a2p", xt2, KC, ACT.Identity, bt["a2b"])
            mu1, ve1 = ln_phase1(t2, "l1")

            e0_blocks(range(12, 16))
            h2 = ipool.tile([P, KC, T], BF16, tag="tC", name="h2")
            ln_phase2(h2, t2, mu1, ve1, "l1")
            out_proj(ys[0], "w2p", h1f, KI, bt["zb"], wbp)

            # ---- expert 2 (part 2): FF + residual + LN2 stats ------------
            g2 = ipool.tile([P, K2, T], BF16, tag="tD", name="g2")
            up_proj(g2, "f1p", h2, KC, ACT.Relu, bt["f1b"])
            ffa = ipool.tile([P, KC, T], BF16, tag="tB", name="ffa")
            nb_f2 = PACKED_W["f2p"][0]
            for b_ in range(nb_f2):
                wc = wsp2.tile([P, K2, 256], BF16, tag="w")
                nc.sync.dma_start(wc[:], wd["f2p"].ap()[b_])
                for ml in range(2):
                    c = 2 * b_ + ml
                    ps = psp.tile([P, T], F32, tag="mm")
                    for kc in range(K2):
                        nc.tensor.matmul(ps[:], wc[:, kc, ml * P:(ml + 1) * P],
                                         g2[:, kc, :],
                                         start=(kc == 0), stop=(kc == K2 - 1))
                    # ffa = ff + f2b + h2   (residual)
                    nc.vector.scalar_tensor_tensor(
                        ffa[:, c, :], ps[:], bt["f2b"][:, c:c + 1],
                        h2[:, c, :], OP.add, OP.add)
            # ---- expert 3 up-proj; LN2 math hidden underneath ------------
            xt3 = load_xg(3)
            mu2, ve2 = ln_phase1(ffa, "l2")
            h1 = hpool.tile([P, KI, T], BF16, tag="h1", name="h1e3")
            up_proj(h1, "m1p", xt3, KC, ACT.Gelu, bt["m1b"],
                    blocks=range(0, 8))
            h2b = ipool.tile([P, KC, T], BF16, tag="tA", name="h2b")
            ln_phase2(h2b, ffa, mu2, ve2, "l2")
            up_proj(h1, "m1p", xt3, KC, ACT.Gelu, bt["m1b"],
                    blocks=range(8, 16))
            out_proj(ys[2], "genp", h2b, KC, bt["genb"], wsp)

            # ---- expert 3 down-projection --------------------------------
            out_proj(ys[3], "m2p", h1, KI, bt["m2b"], wbp, evict_eng=nc.sync)

    nc.compile()
    return nc


_PROGRAM = None


def _get_program():
    global _PROGRAM
    if _PROGRAM is None:
        _PROGRAM = build_moe_sparse()
    return _PROGRAM


def run_cores(nc, in_maps, trace=False, trace_cores=None):
    if trace:
        _install_ntff_shim()
    return run_bass_kernel_spmd(nc, in_maps, core_ids=list(range(len(in_maps))),
                                trace=trace, trace_cores=trace_cores)


# ---- host side ---------------------------------------------------------
def _gelu(x):
    try:
        from scipy.special import erf
        return 0.5 * x * (1.0 + erf(x / math.sqrt(2.0)))
    except ImportError:
        ve = np.vectorize(math.erf)
        return 0.5 * x * (1.0 + ve(x / math.sqrt(2.0)))


def _ln64(h, g, b, eps=1e-5):
    mu = h.mean(-1, keepdims=True)
    var = ((h - mu) ** 2).mean(-1, keepdims=True)
    return (h - mu) / np.sqrt(var + eps) * g + b


def _pack_w(w, kcc, dt=None):
    """[K, M] fp64 -> [M//256, P, kcc, 256] contiguous tile blocks."""
    K, M = w.shape
    assert K == kcc * P
    r = w.reshape(kcc, P, M)
    blocks = [np.ascontiguousarray(r[:, :, b * 256:(b + 1) * 256]
                                   .transpose(1, 0, 2))
              for b in range(M // 256)]
    return np.stack(blocks, 0).astype(dt or BF)


def _p2scale(a, target=96.0):
    """Power-of-2 scale putting |a|max near `target` (e4m3 max is 240)."""
    am = float(np.abs(a).max())
    if am == 0.0:
        return 1.0
    return 2.0 ** round(math.log2(target / am))


def _pack_b(b):
    n = b.shape[0] // P
    return np.ascontiguousarray(b.reshape(n, P).T.astype(np.float32))


def prepare(inputs):
    f64 = lambda n: np.asarray(inputs[n], np.float64)
    x = np.asarray(inputs["x"], np.float32).reshape(-1, H)

    # the device LN path hardcodes gamma=1, beta=0 (the reference always
    # passes ones/zeros); verify that assumption on the actual inputs
    assert np.allclose(np.asarray(inputs["ce_n1g"]), 1.0), "ce_n1g != 1"
    assert np.allclose(np.asarray(inputs["ce_n1b"]), 0.0), "ce_n1b != 0"
    assert np.allclose(np.asarray(inputs["ce_n2g"]), 1.0), "ce_n2g != 1"
    assert np.allclose(np.asarray(inputs["ce_n2b"]), 0.0), "ce_n2b != 0"

    # routing (host, fp64)
    lg = x.astype(np.float64) @ f64("router_w")
    lg += f64("router_b") + f64("load_balancer")
    sel = np.argsort(-lg, axis=1, kind="stable")[:, :2]
    ls = np.take_along_axis(lg, sel, 1)
    ew = np.exp(ls - ls.max(1, keepdims=True))
    gates = ew / ew.sum(1, keepdims=True)

    # folded weights (fp64)
    F = {}
    F["A1"] = f64("me_eq_w") @ f64("me_wv") @ f64("me_wo")
    F["a1"] = (f64("me_eq_b") @ f64("me_wv") + f64("me_bv")) @ f64("me_wo") \
        + f64("me_bo")
    W2o = f64("ce_wv") @ f64("ce_wo")
    F["A2"] = f64("ce_syn_w") + f64("ce_syn_w") @ W2o
    F["a2"] = f64("ce_syn_b") + f64("ce_syn_b") @ W2o + f64("ce_bv") \
        @ f64("ce_wo") + f64("ce_bo")

    C1 = F["A1"] @ f64("me_c1w")
    # fp8 scaling (power-of-2; weight absmax exact, activation absmax
    # estimated on a token subsample with a 4x saturation margin)
    sx1 = _p2scale(x)
    sc1 = _p2scale(C1)
    sc2 = _p2scale(f64("me_c2w"))
    xs = x[::16].astype(np.float32)
    a_ = xs @ np.asarray(inputs["sw_w1"], np.float32)
    h1s = a_ / (1.0 + np.exp(-a_)) * (xs @ np.asarray(inputs["sw_w3"],
                                                     np.float32))
    sh = _p2scale(h1s, target=48.0)
    sw2p = _p2scale(f64("sw_w2"))
    wmap = {
        "w1p": (f64("sw_w1"), KC), "w3p": (f64("sw_w3"), KC),
        "w2p": (f64("sw_w2") * sw2p, KI),
        "c1p": (C1 * sc1, KC),
        "c2p": (f64("me_c2w") * sc2, K2),
        "a2p": (F["A2"], KC), "f1p": (f64("ce_f1w"), KC),
        "f2p": (f64("ce_f2w"), K2), "genp": (f64("ce_gen_w"), KC),
        "m1p": (f64("ml_w1"), KC), "m2p": (f64("ml_w2"), KI),
    }
    bmap = {
        "c1b": F["a1"] @ f64("me_c1w") + f64("me_c1b"),
        "c2b": f64("me_c2b"),
        "a2b": F["a2"],
        "f1b": f64("ce_f1b"), "f2b": f64("ce_f2b"),
        "n1g": f64("ce_n1g"), "n1b": f64("ce_n1b"),
        "n2g": f64("ce_n2g"), "n2b": f64("ce_n2b"),
        "genb": f64("ce_gen_b"), "m1b": f64("ml_b1"), "m2b": f64("ml_b2"),
        "zb": np.zeros(H),
        "c1s": np.full(P, 1.0 / (sx1 * sc1)),
        "c2s": np.full(P, 1.0 / sc2),
        "w2s": np.full(P, 1.0 / (sh * sw2p)),
        "hsc": np.full(P, sh),
    }
    base = {n: _pack_w(w, kcc, F8 if n in FP8_W else None)
            for n, (w, kcc) in wmap.items()}
    base["ball"] = np.concatenate([_pack_b(bmap[n]) for n in BIASES], 1)

    meta = {"x": x, "gates": gates, "sel": sel, "F": F,
            "dev_idx": [], "dev_w": [], "ovf": []}
    in_maps = [dict(base) for _ in range(N_CORES)]
    for e in range(E):
        m = sel == e
        tok = np.nonzero(m.any(1))[0]
        we = np.where(m[:, 0][tok], gates[tok, 0], gates[tok, 1])
        dev, ovf = tok[:CAP], tok[CAP:]
        meta["dev_idx"].append(dev)
        meta["dev_w"].append(we[:len(dev)])
        meta["ovf"].append((ovf, we[len(dev):]))
        xfull = np.zeros((CAP, H), np.float32)
        xfull[:len(dev)] = x[dev]
        percore = xfull.reshape(N_CORES, T, H)
        for c in range(N_CORES):
            xc = percore[c].T.reshape(KC, P, T).transpose(1, 0, 2)
            if e == 1:
                in_maps[c][f"xg{e}"] = np.ascontiguousarray(
                    xc * sx1).astype(F8)
            else:
                in_maps[c][f"xg{e}"] = np.ascontiguousarray(xc).astype(BF)
    meta["in_maps"] = in_maps
    return meta


def _host_expert(e, xs, inputs, F):
    """Overflow tokens, fp64, replicating the reference formulas."""
    f64 = lambda n: np.asarray(inputs[n], np.float64)
    xs = xs.astype(np.float64)
    if e == 0:
        a = xs @ f64("sw_w1")
        g = a / (1.0 + np.exp(-a)) * (xs @ f64("sw_w3"))
        return g @ f64("sw_w2")
    if e == 1:
        t = xs @ F["A1"] + F["a1"]
        g = _gelu(t @ f64("me_c1w") + f64("me_c1b"))
        return g @ f64("me_c2w") + f64("me_c2b")
    if e == 2:
        t = xs @ F["A2"] + F["a2"]
        h2 = _ln64(t, f64("ce_n1g"), f64("ce_n1b"))
        ff = np.maximum(h2 @ f64("ce_f1w") + f64("ce_f1b"), 0.0) \
            @ f64("ce_f2w") + f64("ce_f2b")
        h2 = _ln64(h2 + ff, f64("ce_n2g"), f64("ce_n2b"))
        return h2 @ f64("ce_gen_w") + f64("ce_gen_b")
    a = _gelu(xs @ f64("ml_w1") + f64("ml_b1"))
    return a @ f64("ml_w2") + f64("ml_b2")


def combine(meta, results, inputs):
    out = np.zeros((B * S, H), np.float32)
    for e in range(E):
        ye = np.concatenate(
            [results[c][f"y{e}"].transpose(2, 1, 0).reshape(T, H)
             for c in range(N_CORES)], 0)
        dev, we = meta["dev_idx"][e], meta["dev_w"][e]
        out[dev] += (we[:, None] * ye[:len(dev)]).astype(np.float32)
        ovf, wo = meta["ovf"][e]
        if len(ovf):
            yh = _host_expert(e, meta["x"][ovf], inputs, meta["F"])
            out[ovf] += (wo[:, None] * yh).astype(np.float32)
    return out.reshape(B, S, H)


def kernel(**inputs):
    nc = _get_program()
    meta = prepare(inputs)
    # transient NRT/axon device errors (UNAVAILABLE / INTERNAL) have been
    # observed on this fleet and clear on re-run: retry a bounded number
    # of times rather than failing the whole call
    last = None
    for _ in range(3):
        try:
            res = run_cores(nc, meta["in_maps"])
            break
        except Exception as e:
            last = e
    else:
        raise last
    return combine(meta, [res.results[c] for c in range(N_CORES)], inputs)


# ---- NTFF profiling shim (axon) — used by test.py only ----------------
def _install_ntff_shim():
    import contextlib
    import ctypes
    import sys
    import types

    if "antenv.axon_hooks" in sys.modules:
        return
    lib = ctypes.CDLL("/opt/axon/libaxon_pjrt.so")
    if not hasattr(lib, "axon_start_nrt_profile"):
        return
    lib.axon_start_nrt_profile.argtypes = [ctypes.POINTER(ctypes.c_int64),
                                           ctypes.c_size_t]
    lib.axon_start_nrt_profile.restype = ctypes.c_int64
    lib.axon_stop_nrt_profile.argtypes = [ctypes.c_char_p]
    lib.axon_stop_nrt_profile.restype = ctypes.c_int64

    @contextlib.contextmanager
    def _hook(output_dir, device_ids):
        import jax
        jax.devices()
        if device_ids:
            ids = (ctypes.c_int64 * len(device_ids))(*device_ids)
            rc = lib.axon_start_nrt_profile(ids, len(device_ids))
        else:
            rc = lib.axon_start_nrt_profile(None, 0)
        if rc != 0:
            raise RuntimeError(f"axon_start_nrt_profile rc={rc}")
        try:
            yield
        finally:
            n = lib.axon_stop_nrt_profile(str(output_dir).encode())
            print(f"profile: {n} file(s) written to {output_dir}",
                  file=sys.stderr)

    import antenv
    mod = types.ModuleType("antenv.axon_hooks")
    mod.get_axon_ntff_profile_hook = lambda: _hook
    mod.set_axon_ntff_profile_hook = lambda hk: None
    sys.modules["antenv.axon_hooks"] = mod
    antenv.axon_hooks = mod


# revision 49
# speedup vs baseline: 1.1946x; 1.0055x over previous
"""Trainium2 Bass kernel for nn_MixtureOfExpertsLayer (moe_routing).

Sparse dispatch: top-2 routing is computed on the host (the router is a
tiny [8192,1024]@[1024,4] GEMM); tokens are gathered per expert and
sharded across the 8 cores so each core runs a fixed 512-token slab
through each of the 4 experts — half the dense FLOPs.  The linear
chains inside experts 1/2 are pre-folded on the host
(eq_w@wv@wo and syn_w@(I+wv@wo)), removing another ~11% of matmul work.

Device compute is bf16 (PSUM accumulates fp32).  Weights are pre-packed
on the host into the exact [p, kc, 256-col] tile layout the tensor
engine wants, so every DMA is a fully contiguous 0.5-2MB block.
Activations stay feature-major [128p, chunk, tok]; every matmul has a
512-token moving dim (full PE rate).  Expert outputs come back
feature-major [H, 512] fp32; the host applies the top-2 softmax gates
and scatter-adds into the final output.  Tokens beyond the
4096-per-expert device capacity (a few dozen when routing is balanced)
are computed on the host in fp64.

PE-roofline tuning (vs the first working version, 447us -> 380us):
 - Expert 1 (both GEMMs) and E0's down-projection run in fp8-e4m3 with
   DoubleRow perf mode (2x matmul rate).  Weights are pre-scaled by a
   power of two on the host (e4m3 normals start at 2^-6, so the 0.02-std
   weights must be scaled up); inverse scales ride in the bias pack and
   are applied by the ACT drain (act(scale*psum+bias)).  Simulated /
   measured output error 0.0146 vs the 2e-2 gate.
 - LayerNorm stats no longer use PE matmuls: per-chunk sums/squares are
   tree-summed on the DVE and reduced across partitions with the
   GpSimd daisy-chain partition_all_reduce (result already broadcast),
   freeing ~8us of tensor-engine time and two PSUM pools.
 - Expert 1 runs FIRST: its fp8 weights halve the startup bytes and its
   DoubleRow pace matches the cold DMA ring, so the PE never starves
   (no HAM re-throttle) while the bf16 experts stream in.  A short
   dummy-matmul burst covers the initial DMA fill.
 - out_proj psum drains run on the ACT engine (idle there) so psum
   release keeps up with the fp8 matmul rate; eviction DMAs issue from
   the GpSimd DGE (sync HWDGE for the final m2p so the tail isn't
   behind the slower GpSimd path); the matmul PSUM pool gets 7 banks.
"""
import math

import numpy as np
import ml_dtypes

import concourse.bass as bass
import concourse.mybir as mybir
import concourse.tile as tile
from concourse import bacc
from concourse.alu_op_type import AluOpType
from concourse.bass_utils import run_bass_kernel_spmd

F32 = mybir.dt.float32
BF16 = mybir.dt.bfloat16
FP8 = mybir.dt.float8e4
ACT = mybir.ActivationFunctionType
AX = mybir.AxisListType
OP = AluOpType
RED = bass.bass_isa.ReduceOp
DR = mybir.MatmulPerfMode.DoubleRow
BF = ml_dtypes.bfloat16
F8 = ml_dtypes.float8_e4m3

N_CORES = 8
B, S, H, I, E = 4, 2048, 1024, 4096, 4
P = 128
T = 512                   # tokens per expert per core
CAP = N_CORES * T         # device capacity per expert
KC = H // P               # 8
KI = I // P               # 32
K2 = (2 * H) // P         # 16

# packed weight dram tensors: name -> (n_256col_blocks, contraction_chunks)
PACKED_W = {
    "w1p": (I // 256, KC), "w3p": (I // 256, KC), "m1p": (I // 256, KC),
    "w2p": (H // 256, KI), "m2p": (H // 256, KI),
    "c1p": (2 * H // 256, KC), "f1p": (2 * H // 256, KC),
    "c2p": (H // 256, K2), "f2p": (H // 256, K2),
    "a2p": (H // 256, KC), "genp": (H // 256, KC),
}
# expert 1 and E0's down-projection run in fp8-e4m3 with DoubleRow
# (simulated total error 0.0147 vs the 2e-2 gate); weights are
# pre-scaled by a power of two on the host and the inverse scales ride
# in the bias pack
FP8_W = {"c1p", "c2p", "w2p"}
FP8_OUT_SCALE = {"c2p": "c2s", "w2p": "w2s"}
# biases live in one packed [P, sum] f32 tensor; name -> n_chunks
BIASES = {
    "c1b": K2, "c2b": KC,
    "a2b": KC, "f1b": K2, "f2b": KC,
    "n1g": KC, "n1b": KC, "n2g": KC, "n2b": KC, "genb": KC,
    "m1b": KI, "m2b": KC, "zb": KC,
    "c1s": 1, "c2s": 1, "w2s": 1, "hsc": 1,
}
BIAS_OFF = {}
_off = 0
for _n, _c in BIASES.items():
    BIAS_OFF[_n] = _off
    _off += _c
BIAS_COLS = _off


def build_moe_sparse():
    nc = bacc.Bacc("TRN2", target_bir_lowering=False, debug=False)

    xg = [nc.dram_tensor(f"xg{e}", [P, KC, T], FP8 if e == 1 else BF16,
                         kind="ExternalInput")
          for e in range(E)]
    wd = {n: nc.dram_tensor(n, [nb, P, kcc, 256],
                            FP8 if n in FP8_W else BF16,
                            kind="ExternalInput")
          for n, (nb, kcc) in PACKED_W.items()}
    ball = nc.dram_tensor("ball", [P, BIAS_COLS], F32, kind="ExternalInput")
    ys = [nc.dram_tensor(f"y{e}", [P, KC, T], F32, kind="ExternalOutput")
          for e in range(E)]

    with tile.TileContext(nc) as tc:
        with (
            tc.tile_pool(name="const", bufs=1) as cpool,
            tc.tile_pool(name="xg", bufs=2) as xpool,
            tc.tile_pool(name="h1", bufs=1) as hpool,
            tc.tile_pool(name="inter", bufs=1) as ipool,
            tc.tile_pool(name="ws", bufs=5) as wsp,     # KC-contraction blocks
            tc.tile_pool(name="ws2", bufs=3) as wsp2,   # K2-contraction blocks
            tc.tile_pool(name="wb", bufs=2) as wbp,     # KI-contraction blocks
            tc.tile_pool(name="yev", bufs=3) as ypool,  # output eviction
            tc.tile_pool(name="lns", bufs=2) as lnsp,   # LN stat tiles
            tc.tile_pool(name="sq", bufs=2) as sqp,     # LN scratch (bf16)
            tc.tile_pool(name="ps", bufs=8, space=bass.MemorySpace.PSUM) as psp,
        ):
            # ---- constants ------------------------------------------------
            ones_c = cpool.tile([P, 1], BF16, tag="ones_c")
            nc.vector.memset(ones_c[:], 1.0)
            dmt = cpool.tile([P, T], BF16, tag="dmt")
            nc.vector.memset(dmt[:], 0.0078125)

            bt_all = cpool.tile([P, BIAS_COLS], F32, tag="ball")
            bt = {n: bt_all[:, BIAS_OFF[n]:BIAS_OFF[n] + nch]
                  for n, nch in BIASES.items()}

            # ---- HAM warm-up: dummy matmuls fill the initial DMA wait ----
            # (they borrow a ring slot from the matmul psum pool so all 8
            # banks are available to real matmuls afterwards)
            wps = psp.tile([P, T], F32, tag="mm", name="wps")[0:1, :]

            def warm(n):
                # dummy matmuls on resident constants: keep the PE HAM
                # activity window fed while the startup DMAs stream in
                for _ in range(n):
                    nc.tensor.matmul(wps[:], ones_c[:], dmt[:],
                                     start=True, stop=True)

            # a short contiguous burst while the first DMAs stream in; the
            # startup is ring-throughput-bound, so the early real matmuls
            # run at the cold-clock pace that matches the ring anyway
            warm(7)

            # E0's intermediate is written directly in scaled fp8 for the
            # w2p DoubleRow down-projection; E3 later reuses the same pool
            # slot for its bf16 intermediate (disjoint lifetimes)
            h1f = hpool.tile([P, KI, T], FP8, tag="h1", name="h1f")
            # bias/scale pack: tiny (36KB) — load first on the idle gpsimd
            # DGE so the fp8 scale columns are resident before E0's first
            # h1 write
            nc.gpsimd.dma_start(bt_all[:], ball.ap())

            # ---- helpers --------------------------------------------------
            def load_xg(e, split=1):
                t_ = xpool.tile([P, KC, T], FP8 if e == 1 else BF16,
                                tag="xg", name=f"xgt{e}")
                step = KC // split
                for i in range(split):
                    sl = slice(i * step, (i + 1) * step)
                    nc.sync.dma_start(t_[:, sl, :], xg[e].ap()[:, sl, :])
                return t_

            def mm_block(ps, wc, src, src_kc, ml, fp8):
                """Accumulate one 256-col block into ps; fp8 runs DoubleRow
                (two contraction chunks per instruction at 2x rate)."""
                if fp8:
                    for kc in range(0, src_kc, 2):
                        nc.tensor.matmul(
                            ps[:], wc[:, kc:kc + 2, ml * P:(ml + 1) * P],
                            src[:, kc:kc + 2, :],
                            start=(kc == 0), stop=(kc == src_kc - 2),
                            perf_mode=DR)
                else:
                    for kc in range(src_kc):
                        nc.tensor.matmul(
                            ps[:], wc[:, kc, ml * P:(ml + 1) * P],
                            src[:, kc, :],
                            start=(kc == 0), stop=(kc == src_kc - 1))

            def up_proj(dst, wname, src, src_kc, act, bias, blocks=None,
                        pool=None, scale=None, w0=None, hook=None):
                """dst[:, c, :] = act(scale * (Wc.T @ src) + bias_c),
                streamed in 256-col blocks.  dst chunk c = 2*b + ml."""
                pool = pool or wsp
                fp8 = wname in FP8_W
                nb = PACKED_W[wname][0]
                for b_ in (range(nb) if blocks is None else blocks):
                    if b_ == 0 and w0 is not None:
                        wc = w0
                    else:
                        wc = pool.tile([P, src_kc, 256],
                                       FP8 if fp8 else BF16, tag="w")
                        nc.sync.dma_start(wc[:], wd[wname].ap()[b_])
                    if hook is not None:
                        hook(b_)
                    for ml in range(2):
                        c = 2 * b_ + ml
                        ps = psp.tile([P, T], F32, tag="mm")
                        mm_block(ps, wc, src, src_kc, ml, fp8)
                        b_sl = None if bias is None else bias[:, c:c + 1]
                        nc.scalar.activation(dst[:, c, :], ps[:], act,
                                             bias=b_sl,
                                             scale=(scale if scale is not None
                                                    else 1.0))

            def out_proj(ydram, wname, src, src_kc, bias, wpool,
                         evict_eng=None, w0=()):
                """y[:, c, :] = Wc.T @ src + bias_c -> DMA to DRAM (fp32).
                Drain via ACT Identity, eviction DMA issued from GpSimd (or
                the given engine — the final out_proj uses the idle Sync
                HWDGE so the tail eviction isn't behind the GpSimd DGE)."""
                evict_eng = evict_eng or nc.gpsimd
                fp8 = wname in FP8_W
                nb = PACKED_W[wname][0]
                for b_ in range(nb):
                    if b_ < len(w0):
                        wc = w0[b_]
                    else:
                        wc = wpool.tile([P, src_kc, 256],
                                        FP8 if fp8 else BF16, tag="w")
                        nc.sync.dma_start(wc[:], wd[wname].ap()[b_])
                    for ml in range(2):
                        c = 2 * b_ + ml
                        ps = psp.tile([P, T], F32, tag="mm")
                        mm_block(ps, wc, src, src_kc, ml, fp8)
                        yt = ypool.tile([P, T], F32, tag="y")
                        # drain on the ACT engine (idle during out_proj) so
                        # the psum-release rate keeps up with fp8 matmuls
                        sc_ = (bt[FP8_OUT_SCALE[wname]][:, 0:1] if fp8
                               else 1.0)
                        nc.scalar.activation(yt[:], ps[:], ACT.Identity,
                                             bias=bias[:, c:c + 1],
                                             scale=sc_)
                        evict_eng.dma_start(ydram.ap()[:, c, :], yt[:])

            # -- LayerNorm (g=1, b=0 verified on host): two-phase, PE-free --
            def ln_phase1(src, tag):
                """mu (bf16) and var+eps (f32), both [P,T] broadcast across
                partitions.  DVE chunk-chains + GpSimd partition reduce."""
                # ssum chain over chunks 0..6 then final add -> f32
                s_acc = None
                for c in range(KC - 2):
                    nxt = sqp.tile([P, T], BF16, tag="tr", name=f"s{tag}{c}")
                    if s_acc is None:
                        nc.vector.tensor_tensor(nxt[:], src[:, 0, :],
                                                src[:, 1, :], OP.add)
                    else:
                        nc.vector.tensor_tensor(nxt[:], s_acc[:],
                                                src[:, c + 1, :], OP.add)
                    s_acc = nxt
                s1f = lnsp.tile([P, T], F32, tag="st", bufs=3,
                                name=f"s1f{tag}")
                nc.vector.tensor_tensor(s1f[:], s_acc[:], src[:, KC - 1, :],
                                        OP.add)
                # ssq: square chunks then chain
                q_acc = None
                for c in range(KC):
                    sqc = sqp.tile([P, T], BF16, tag="sq", name=f"q{tag}{c}")
                    nc.vector.tensor_tensor(sqc[:], src[:, c, :],
                                            src[:, c, :], OP.mult)
                    if c == 1:
                        nxt = sqp.tile([P, T], BF16, tag="qp",
                                       name=f"qa{tag}{c}")
                        nc.vector.tensor_tensor(nxt[:], prev_sq[:], sqc[:],
                                                OP.add)
                        q_acc = nxt
                    elif c > 1 and c < KC - 1:
                        nxt = sqp.tile([P, T], BF16, tag="qp",
                                       name=f"qa{tag}{c}")
                        nc.vector.tensor_tensor(nxt[:], q_acc[:], sqc[:],
                                                OP.add)
                        q_acc = nxt
                    elif c == KC - 1:
                        q1f = lnsp.tile([P, T], F32, tag="st", bufs=3,
                                        name=f"q1f{tag}")
                        nc.vector.tensor_tensor(q1f[:], q_acc[:], sqc[:],
                                                OP.add)
                    prev_sq = sqc
                # cross-partition all-reduce (result broadcast to all parts)
                S_ = lnsp.tile([P, T], F32, tag="st", bufs=3, name=f"S{tag}")
                nc.gpsimd.partition_all_reduce(S_[:], s1f[:], P, RED.add)
                Q_ = lnsp.tile([P, T], F32, tag="st", bufs=3, name=f"Q{tag}")
                nc.gpsimd.partition_all_reduce(Q_[:], q1f[:], P, RED.add)
                # stats math (DVE, f32)
                mu_f = lnsp.tile([P, T], F32, tag="sc", bufs=2,
                                 name=f"muf{tag}")
                nc.vector.tensor_scalar(mu_f[:], S_[:], 1.0 / H, None,
                                        OP.mult)
                mu_b = lnsp.tile([P, T], BF16, tag="mb", bufs=1,
                                 name=f"mub{tag}")
                nc.vector.tensor_copy(mu_b[:], mu_f[:])
                s2 = lnsp.tile([P, T], F32, tag="sc", bufs=2, name=f"s2{tag}")
                nc.vector.tensor_tensor(s2[:], mu_f[:], mu_f[:], OP.mult)
                q1h = lnsp.tile([P, T], F32, tag="sc", bufs=2,
                                name=f"q1h{tag}")
                nc.vector.tensor_scalar(q1h[:], Q_[:], 1.0 / H, None, OP.mult)
                var_eps = lnsp.tile([P, T], F32, tag="ve", bufs=1,
                                    name=f"ve{tag}")
                nc.vector.scalar_tensor_tensor(var_eps[:], q1h[:], 1e-5,
                                               s2[:], OP.add, OP.subtract)
                return mu_b, var_eps

            def ln_phase2(dst, src, mu_b, var_eps, tag):
                """dst = (src - mu) * rsqrt(var+eps)  (bf16; rstd via ACT)."""
                sdev = lnsp.tile([P, T], F32, tag="sd", bufs=1,
                                 name=f"sd{tag}")
                nc.scalar.activation(sdev[:], var_eps[:], ACT.Sqrt)
                rs_b = lnsp.tile([P, T], BF16, tag="rb", bufs=1,
                                 name=f"rsb{tag}")
                with nc.allow_low_precision(reason="rstd in bf16 is ~0.1% "
                                            "rms; well inside tolerance"):
                    nc.vector.reciprocal(rs_b[:], sdev[:])
                for kc in range(KC):
                    t_ = sqp.tile([P, T], BF16, tag="tr", name=f"n{tag}{kc}")
                    nc.vector.tensor_tensor(t_[:], src[:, kc, :], mu_b[:],
                                            OP.subtract)
                    nc.vector.tensor_tensor(dst[:, kc, :], t_[:], rs_b[:],
                                            OP.mult)

            # ---- expert 1 first: its fp8 weights are half the startup ----
            # bytes and the DoubleRow pace matches the cold DMA ring, so
            # the PE never starves while the rest of the model streams in
            xt1 = xpool.tile([P, KC, T], FP8, tag="xg", name="xgt1")
            wc10 = wsp.tile([P, KC, 256], FP8, tag="w", name="wc10")
            nc.sync.dma_start(xt1[:, 0:4, :], xg[1].ap()[:, 0:4, :])
            nc.scalar.dma_start(wc10[:], wd["c1p"].ap()[0])
            nc.sync.dma_start(xt1[:, 4:8, :], xg[1].ap()[:, 4:8, :])

            # prefetch tiles with their own tags so the ring-slot reuse of
            # the main weight pools isn't disturbed
            wc20 = wsp2.tile([P, K2, 256], FP8, tag="wc2", bufs=1,
                             name="wc20")
            wa0e = wsp.tile([P, KC, 256], BF16, tag="w0e", bufs=1,
                            name="wa0e")

            def c1p_hook(b_):
                # pull the next phases' first weight blocks ahead in the
                # sync DMA queue so the c2p / E0 transitions don't starve
                if b_ == 3:
                    nc.sync.dma_start(wc20[:], wd["c2p"].ap()[0])
                elif b_ == 6:
                    nc.sync.dma_start(wa0e[:], wd["w1p"].ap()[0])

            g1 = ipool.tile([P, K2, T], FP8, tag="tD", name="g1")
            up_proj(g1, "c1p", xt1, KC, ACT.Gelu, bt["c1b"],
                    scale=bt["c1s"][:, 0:1], w0=wc10, hook=c1p_hook)
            out_proj(ys[1], "c2p", g1, K2, bt["c2b"], wsp2,
                     w0=[wc20])

            # ---- expert 0 up-proj, with E2's folded front (a2p) tucked ---
            # between its last blocks so a2p's bf16 weights load outside
            # the ring-tight startup window
            xt0 = load_xg(0)

            def e0_blocks(blocks):
                for b_ in blocks:
                    if b_ == 0:
                        wa = wa0e
                    else:
                        wa = wsp.tile([P, KC, 256], BF16, tag="w")
                        nc.sync.dma_start(wa[:], wd["w1p"].ap()[b_])
                    wb = wsp.tile([P, KC, 256], BF16, tag="w")
                    nc.sync.dma_start(wb[:], wd["w3p"].ap()[b_])
                    for ml in range(2):
                        c = 2 * b_ + ml
                        psa = psp.tile([P, T], F32, tag="mm")
                        psb = psp.tile([P, T], F32, tag="mm")
                        for kc in range(KC):
                            nc.tensor.matmul(
                                psa[:], wa[:, kc, ml * P:(ml + 1) * P],
                                xt0[:, kc, :],
                                start=(kc == 0), stop=(kc == KC - 1))
                        for kc in range(KC):
                            nc.tensor.matmul(
                                psb[:], wb[:, kc, ml * P:(ml + 1) * P],
                                xt0[:, kc, :],
                                start=(kc == 0), stop=(kc == KC - 1))
                        sa = ypool.tile([P, T], BF16, tag="sa")
                        nc.scalar.activation(sa[:], psa[:], ACT.Silu)
                        nc.vector.scalar_tensor_tensor(
                            h1f[:, c, :], psb[:], bt["hsc"][:, 0:1], sa[:],
                            OP.mult, OP.mult)

            e0_blocks(range(0, 12))

            # ---- expert 2 (part 1): folded front + LN1 stats -------------
            xt2 = load_xg(2)
            t2 = ipool.tile([P, KC, T], BF16, tag="tA", name="t2")
            up_proj(t2, "a2p", xt2, KC, ACT.Identity, bt["a2b"])
            mu1, ve1 = ln_phase1(t2, "l1")

            e0_blocks(range(12, 16))
            h2 = ipool.tile([P, KC, T], BF16, tag="tC", name="h2")
            ln_phase2(h2, t2, mu1, ve1, "l1")
            out_proj(ys[0], "w2p", h1f, KI, bt["zb"], wbp)

            # ---- expert 2 (part 2): FF + residual + LN2 stats ------------
            g2 = ipool.tile([P, K2, T], BF16, tag="tD", name="g2")
            up_proj(g2, "f1p", h2, KC, ACT.Relu, bt["f1b"])
            ffa = ipool.tile([P, KC, T], BF16, tag="tB", name="ffa")
            nb_f2 = PACKED_W["f2p"][0]
            for b_ in range(nb_f2):
                wc = wsp2.tile([P, K2, 256], BF16, tag="w")
                nc.sync.dma_start(wc[:], wd["f2p"].ap()[b_])
                for ml in range(2):
                    c = 2 * b_ + ml
                    ps = psp.tile([P, T], F32, tag="mm")
                    for kc in range(K2):
                        nc.tensor.matmul(ps[:], wc[:, kc, ml * P:(ml + 1) * P],
                                         g2[:, kc, :],
                                         start=(kc == 0), stop=(kc == K2 - 1))
                    # ffa = ff + f2b + h2   (residual)
                    nc.vector.scalar_tensor_tensor(
                        ffa[:, c, :], ps[:], bt["f2b"][:, c:c + 1],
                        h2[:, c, :], OP.add, OP.add)
            # ---- expert 3 up-proj; LN2 math hidden underneath ------------
            xt3 = load_xg(3)
            mu2, ve2 = ln_phase1(ffa, "l2")
            h1 = hpool.tile([P, KI, T], BF16, tag="h1", name="h1e3")
            up_proj(h1, "m1p", xt3, KC, ACT.Gelu, bt["m1b"],
                    blocks=range(0, 8))
            h2b = ipool.tile([P, KC, T], BF16, tag="tA", name="h2b")
            ln_phase2(h2b, ffa, mu2, ve2, "l2")
            up_proj(h1, "m1p", xt3, KC, ACT.Gelu, bt["m1b"],
                    blocks=range(8, 16))
            out_proj(ys[2], "genp", h2b, KC, bt["genb"], wsp)

            # ---- expert 3 down-projection --------------------------------
            out_proj(ys[3], "m2p", h1, KI, bt["m2b"], wbp, evict_eng=nc.sync)

    nc.compile()
    return nc


_PROGRAM = None


def _get_program():
    global _PROGRAM
    if _PROGRAM is None:
        _PROGRAM = build_moe_sparse()
    return _PROGRAM


def run_cores(nc, in_maps, trace=False, trace_cores=None):
    if trace:
        _install_ntff_shim()
    return run_bass_kernel_spmd(nc, in_maps, core_ids=list(range(len(in_maps))),
                                trace=trace, trace_cores=trace_cores)


# ---- host side ---------------------------------------------------------
def _gelu(x):
    try:
        from scipy.special import erf
        return 0.5 * x * (1.0 + erf(x / math.sqrt(2.0)))
    except ImportError:
        ve = np.vectorize(math.erf)
        return 0.5 * x * (1.0 + ve(x / math.sqrt(2.0)))


def _ln64(h, g, b, eps=1e-5):
    mu = h.mean(-1, keepdims=True)
    var = ((h - mu) ** 2).mean(-1, keepdims=True)
    return (h - mu) / np.sqrt(var + eps) * g + b


def _pack_w(w, kcc, dt=None):
    """[K, M] fp64 -> [M//256, P, kcc, 256] contiguous tile blocks."""
    K, M = w.shape
    assert K == kcc * P
    r = w.reshape(kcc, P, M)
    blocks = [np.ascontiguousarray(r[:, :, b * 256:(b + 1) * 256]
                                   .transpose(1, 0, 2))
              for b in range(M // 256)]
    return np.stack(blocks, 0).astype(dt or BF)


def _p2scale(a, target=96.0):
    """Power-of-2 scale putting |a|max near `target` (e4m3 max is 240)."""
    am = float(np.abs(a).max())
    if am == 0.0:
        return 1.0
    return 2.0 ** round(math.log2(target / am))


def _pack_b(b):
    n = b.shape[0] // P
    return np.ascontiguousarray(b.reshape(n, P).T.astype(np.float32))


def prepare(inputs):
    f64 = lambda n: np.asarray(inputs[n], np.float64)
    x = np.asarray(inputs["x"], np.float32).reshape(-1, H)

    # the device LN path hardcodes gamma=1, beta=0 (the reference always
    # passes ones/zeros); verify that assumption on the actual inputs
    assert np.allclose(np.asarray(inputs["ce_n1g"]), 1.0), "ce_n1g != 1"
    assert np.allclose(np.asarray(inputs["ce_n1b"]), 0.0), "ce_n1b != 0"
    assert np.allclose(np.asarray(inputs["ce_n2g"]), 1.0), "ce_n2g != 1"
    assert np.allclose(np.asarray(inputs["ce_n2b"]), 0.0), "ce_n2b != 0"

    # routing (host, fp64)
    lg = x.astype(np.float64) @ f64("router_w")
    lg += f64("router_b") + f64("load_balancer")
    sel = np.argsort(-lg, axis=1, kind="stable")[:, :2]
    ls = np.take_along_axis(lg, sel, 1)
    ew = np.exp(ls - ls.max(1, keepdims=True))
    gates = ew / ew.sum(1, keepdims=True)

    # folded weights (fp64)
    F = {}
    F["A1"] = f64("me_eq_w") @ f64("me_wv") @ f64("me_wo")
    F["a1"] = (f64("me_eq_b") @ f64("me_wv") + f64("me_bv")) @ f64("me_wo") \
        + f64("me_bo")
    W2o = f64("ce_wv") @ f64("ce_wo")
    F["A2"] = f64("ce_syn_w") + f64("ce_syn_w") @ W2o
    F["a2"] = f64("ce_syn_b") + f64("ce_syn_b") @ W2o + f64("ce_bv") \
        @ f64("ce_wo") + f64("ce_bo")

    C1 = F["A1"] @ f64("me_c1w")
    # fp8 scaling (power-of-2; weight absmax exact, activation absmax
    # estimated on a token subsample with a 4x saturation margin)
    sx1 = _p2scale(x)
    sc1 = _p2scale(C1)
    sc2 = _p2scale(f64("me_c2w"))
    xs = x[::16].astype(np.float32)
    a_ = xs @ np.asarray(inputs["sw_w1"], np.float32)
    h1s = a_ / (1.0 + np.exp(-a_)) * (xs @ np.asarray(inputs["sw_w3"],
                                                     np.float32))
    sh = _p2scale(h1s, target=48.0)
    sw2p = _p2scale(f64("sw_w2"))
    wmap = {
        "w1p": (f64("sw_w1"), KC), "w3p": (f64("sw_w3"), KC),
        "w2p": (f64("sw_w2") * sw2p, KI),
        "c1p": (C1 * sc1, KC),
        "c2p": (f64("me_c2w") * sc2, K2),
        "a2p": (F["A2"], KC), "f1p": (f64("ce_f1w"), KC),
        "f2p": (f64("ce_f2w"), K2), "genp": (f64("ce_gen_w"), KC),
        "m1p": (f64("ml_w1"), KC), "m2p": (f64("ml_w2"), KI),
    }
    bmap = {
        "c1b": F["a1"] @ f64("me_c1w") + f64("me_c1b"),
        "c2b": f64("me_c2b"),
        "a2b": F["a2"],
        "f1b": f64("ce_f1b"), "f2b": f64("ce_f2b"),
        "n1g": f64("ce_n1g"), "n1b": f64("ce_n1b"),
        "n2g": f64("ce_n2g"), "n2b": f64("ce_n2b"),
        "genb": f64("ce_gen_b"), "m1b": f64("ml_b1"), "m2b": f64("ml_b2"),
        "zb": np.zeros(H),
        "c1s": np.full(P, 1.0 / (sx1 * sc1)),
        "c2s": np.full(P, 1.0 / sc2),
        "w2s": np.full(P, 1.0 / (sh * sw2p)),
        "hsc": np.full(P, sh),
    }
    base = {n: _pack_w(w, kcc, F8 if n in FP8_W else None)
            for n, (w, kcc) in wmap.items()}
    base["ball"] = np.concatenate([_pack_b(bmap[n]) for n in BIASES], 1)

    meta = {"x": x, "gates": gates, "sel": sel, "F": F,
            "dev_idx": [], "dev_w": [], "ovf": []}
    in_maps = [dict(base) for _ in range(N_CORES)]
    for e in range(E):
        m = sel == e
        tok = np.nonzero(m.any(1))[0]
        we = np.where(m[:, 0][tok], gates[tok, 0], gates[tok, 1])
        dev, ovf = tok[:CAP], tok[CAP:]
        meta["dev_idx"].append(dev)
        meta["dev_w"].append(we[:len(dev)])
        meta["ovf"].append((ovf, we[len(dev):]))
        xfull = np.zeros((CAP, H), np.float32)
        xfull[:len(dev)] = x[dev]
        percore = xfull.reshape(N_CORES, T, H)
        for c in range(N_CORES):
            xc = percore[c].T.reshape(KC, P, T).transpose(1, 0, 2)
            if e == 1:
                in_maps[c][f"xg{e}"] = np.ascontiguousarray(
                    xc * sx1).astype(F8)
            else:
                in_maps[c][f"xg{e}"] = np.ascontiguousarray(xc).astype(BF)
    meta["in_maps"] = in_maps
    return meta


def _host_expert(e, xs, inputs, F):
    """Overflow tokens, fp64, replicating the reference formulas."""
    f64 = lambda n: np.asarray(inputs[n], np.float64)
    xs = xs.astype(np.float64)
    if e == 0:
        a = xs @ f64("sw_w1")
        g = a / (1.0 + np.exp(-a)) * (xs @ f64("sw_w3"))
        return g @ f64("sw_w2")
    if e == 1:
        t = xs @ F["A1"] + F["a1"]
        g = _gelu(t @ f64("me_c1w") + f64("me_c1b"))
        return g @ f64("me_c2w") + f64("me_c2b")
    if e == 2:
        t = xs @ F["A2"] + F["a2"]
        h2 = _ln64(t, f64("ce_n1g"), f64("ce_n1b"))
        ff = np.maximum(h2 @ f64("ce_f1w") + f64("ce_f1b"), 0.0) \
            @ f64("ce_f2w") + f64("ce_f2b")
        h2 = _ln64(h2 + ff, f64("ce_n2g"), f64("ce_n2b"))
        return h2 @ f64("ce_gen_w") + f64("ce_gen_b")
    a = _gelu(xs @ f64("ml_w1") + f64("ml_b1"))
    return a @ f64("ml_w2") + f64("ml_b2")


def combine(meta, results, inputs):
    out = np.zeros((B * S, H), np.float32)
    for e in range(E):
        ye = np.concatenate(
            [results[c][f"y{e}"].transpose(2, 1, 0).reshape(T, H)
             for c in range(N_CORES)], 0)
        dev, we = meta["dev_idx"][e], meta["dev_w"][e]
        out[dev] += (we[:, None] * ye[:len(dev)]).astype(np.float32)
        ovf, wo = meta["ovf"][e]
        if len(ovf):
            yh = _host_expert(e, meta["x"][ovf], inputs, meta["F"])
            out[ovf] += (wo[:, None] * yh).astype(np.float32)
    return out.reshape(B, S, H)


def kernel(**inputs):
    nc = _get_program()
    meta = prepare(inputs)
    # transient NRT/axon device errors (UNAVAILABLE / INTERNAL) have been
    # observed on this fleet and clear on re-run: retry a bounded number
    # of times rather than failing the whole call
    last = None
    for _ in range(3):
        try:
            res = run_cores(nc, meta["in_maps"])
            break
        except Exception as e:
            last = e
    else:
        raise last
    return combine(meta, [res.results[c] for c in range(N_CORES)], inputs)


# ---- NTFF profiling shim (axon) — used by test.py only ----------------
def _install_ntff_shim():
    import contextlib
    import ctypes
    import sys
    import types

    if "antenv.axon_hooks" in sys.modules:
        return
    lib = ctypes.CDLL("/opt/axon/libaxon_pjrt.so")
    if not hasattr(lib, "axon_start_nrt_profile"):
        return
    lib.axon_start_nrt_profile.argtypes = [ctypes.POINTER(ctypes.c_int64),
                                           ctypes.c_size_t]
    lib.axon_start_nrt_profile.restype = ctypes.c_int64
    lib.axon_stop_nrt_profile.argtypes = [ctypes.c_char_p]
    lib.axon_stop_nrt_profile.restype = ctypes.c_int64

    @contextlib.contextmanager
    def _hook(output_dir, device_ids):
        import jax
        jax.devices()
        if device_ids:
            ids = (ctypes.c_int64 * len(device_ids))(*device_ids)
            rc = lib.axon_start_nrt_profile(ids, len(device_ids))
        else:
            rc = lib.axon_start_nrt_profile(None, 0)
        if rc != 0:
            raise RuntimeError(f"axon_start_nrt_profile rc={rc}")
        try:
            yield
        finally:
            n = lib.axon_stop_nrt_profile(str(output_dir).encode())
            print(f"profile: {n} file(s) written to {output_dir}",
                  file=sys.stderr)

    import antenv
    mod = types.ModuleType("antenv.axon_hooks")
    mod.get_axon_ntff_profile_hook = lambda: _hook
    mod.set_axon_ntff_profile_hook = lambda hk: None
    sys.modules["antenv.axon_hooks"] = mod
    antenv.axon_hooks = mod
